# revision 2
# baseline (speedup 1.0000x reference)
"""Llama decoder layer on 8 Trainium2 NeuronCores.

Strategy (sequence-parallel, transposed activation space):
  - Activations live transposed on-chip: [feature, token]; contraction dims land
    on SBUF partitions so every matmul is a natural PE op.
  - Each core owns S/8 = 256 token columns end-to-end (attention rows + full-FF MLP).
  - K/V projections are head-sharded (4 heads per core over all 2048 tokens),
    RoPE'd locally, then AllGathered so each core attends over all 32 heads for
    its 256 query columns. That is the only cross-core communication.
  - Matmuls run in float32r (full-rate PE, ~1.6e-4 rel err); norms/residual/rope
    combine in fp32.
"""
import os
import sys

sys.path.insert(0, "/opt/trn_rl_repo")

import numpy as np

import concourse.bacc as bacc
import concourse.mybir as mybir
import concourse.tile as tile
from concourse.bass_utils import run_bass_kernel_spmd

F32 = mybir.dt.float32
F32R = mybir.dt.float32r
BF16 = mybir.dt.bfloat16
AF = mybir.ActivationFunctionType
MUL = mybir.AluOpType.mult
ADD = mybir.AluOpType.add

B, S, D, H = 1, 2048, 2048, 32
HD = D // H            # 64
FF = 8192
EPS = 1e-5
THETA = 10000.0
P = 128
NCORES = 8
SL = S // NCORES       # 256 local tokens
DL = D // NCORES       # 256 local K/V dims (4 heads)
KD = D // P            # 16 chunks of D
KF = FF // P           # 64 chunks of FF
NPAIR = H // 2         # 16 head pairs

MM_DT = F32R           # matmul compute dtype
MLP_DT = BF16          # MLP weights/activations (halves the dominant DMA traffic)


def _build(single_core=False):
    nc = bacc.Bacc(None, target_bir_lowering=False,
                   num_devices=1 if single_core else NCORES)

    # ---- I/O ----
    # x^T tiled [p, kc, s]; typed f32r so the tile it lands in can feed matmuls
    # directly (the verifier requires every writer of a matmul operand to be
    # f32r-typed, which rules out the in-place f32 load + bitcast approach)
    xt = nc.dram_tensor("xt", [P, KD, S], MM_DT, kind="ExternalInput")
    xl = nc.dram_tensor("xl", [P, KD, SL], F32, kind="ExternalInput")      # local x^T columns
    wq = nc.dram_tensor("wq", [H, P, KD, HD], MM_DT, kind="ExternalInput")  # per-head [h, p, kc, f]
    wk = nc.dram_tensor("wk", [2, P, KD, P], MM_DT, kind="ExternalInput")
    wv = nc.dram_tensor("wv", [P, KD, DL], MM_DT, kind="ExternalInput")     # rhs layout
    wo = nc.dram_tensor("wo", [KD, P, KD, P], MM_DT, kind="ExternalInput")
    wg = nc.dram_tensor("wg", [KF, P, KD, P], MLP_DT, kind="ExternalInput")
    wu = nc.dram_tensor("wu", [KF, P, KD, P], MLP_DT, kind="ExternalInput")
    wd = nc.dram_tensor("wd", [KD, P, KF, P], MLP_DT, kind="ExternalInput")
    cosf = nc.dram_tensor("cosf", [P, S], F32, kind="ExternalInput")       # K rope tables
    sinf = nc.dram_tensor("sinf", [P, S], F32, kind="ExternalInput")
    cosl = nc.dram_tensor("cosl", [P, SL], F32, kind="ExternalInput")      # Q rope tables (local)
    sinl = nc.dram_tensor("sinl", [P, SL], F32, kind="ExternalInput")
    perm = nc.dram_tensor("perm", [P, P], F32, kind="ExternalInput")       # rotate_half permutation
    out = nc.dram_tensor("out", [D, SL], F32, kind="ExternalOutput")       # out^T local columns

    with tile.TileContext(nc) as tc:
        with (
            tc.tile_pool(name="persist", bufs=1) as persist,
            tc.tile_pool(name="dram", bufs=1, space="DRAM") as dram,
        ):
            ones_f = persist.tile([P, 1], F32)
            nc.vector.memset(ones_f, 1.0)
            ones_bf = persist.tile([P, 1], BF16)
            nc.vector.tensor_copy(ones_bf, ones_f)
            ones_r = persist.tile([P, 1], MM_DT)
            nc.vector.tensor_copy(ones_r, ones_f)
            perm_t = persist.tile([P, P], F32)
            nc.sync.dma_start(perm_t, perm[:, :])
            eps_t = persist.tile([P, 1], F32)
            nc.vector.memset(eps_t, EPS)
            # roped Q^T per head on partitions 0:63 (base-64 matmul operands hang
            # at runtime, so everything head-level stays at partition base 0)
            qTr = persist.tile([HD, H, SL], MM_DT)

            k_in = dram.tile([DL, S], MM_DT)
            v_in = dram.tile([S, DL], MM_DT)
            kv_addr = "Local" if single_core else "Shared"
            k_out = dram.tile([NCORES * DL, S], MM_DT, addr_space=kv_addr)
            v_out = dram.tile([NCORES * S, DL], MM_DT, addr_space=kv_addr)

            # ================= phase 1: full in-norm, h^T = x^T * rstd =================
            with (
                tc.tile_pool(name="p1", bufs=1) as p1,
                tc.tile_pool(name="ps1", bufs=1, space="PSUM") as ps1,
            ):
                xh = p1.tile([P, KD, S], MM_DT)   # x^T, becomes h^T in place
                for i in range(4):
                    nc.sync.dma_start(xh[:, 4 * i:4 * (i + 1)], xt[:, 4 * i:4 * (i + 1)])

                with tc.tile_pool(name="p1b", bufs=1) as p1b:
                    ps_ms = [ps1.tile([1, 512], F32, name=f"ps_ms{n}") for n in range(4)]
                    for kc in range(KD):
                        sq = p1b.tile([P, S], BF16, tag="sq", bufs=2)
                        nc.vector.tensor_tensor(sq, xh[:, kc], xh[:, kc], MUL)
                        for n in range(4):
                            nc.tensor.matmul(ps_ms[n], ones_bf, sq[:, 512 * n:512 * (n + 1)],
                                             start=(kc == 0), stop=(kc == KD - 1))
                    rstd = p1b.tile([1, S], F32)
                    for n in range(4):
                        nc.scalar.activation(rstd[:, 512 * n:512 * (n + 1)], ps_ms[n],
                                             AF.Identity, bias=eps_t[0:1], scale=1.0 / D)
                    nc.scalar.sqrt(rstd, rstd)
                    nc.vector.reciprocal(rstd, rstd)
                    rbc_f = p1b.tile([P, S], F32)
                    nc.gpsimd.partition_broadcast(rbc_f, rstd)
                    rbc = p1b.tile([P, S], MM_DT)
                    nc.vector.tensor_copy(rbc, rbc_f)
                    for kc in range(KD):
                        nc.vector.tensor_tensor(xh[:, kc], xh[:, kc], rbc, MUL)
                h = xh

                # ================= phase 2: K projection + rope -> k_in =================
                with (
                    tc.tile_pool(name="p2", bufs=1) as p2,
                    tc.tile_pool(name="ps2", bufs=2, space="PSUM") as ps2,
                ):
                    cosf_t = p2.tile([P, S], F32)
                    sinf_t = p2.tile([P, S], F32)
                    nc.sync.dma_start(cosf_t, cosf[:, :])
                    nc.sync.dma_start(sinf_t, sinf[:, :])
                    for m in range(2):
                        wk_t = p2.tile([P, KD, P], MM_DT, tag="wk", bufs=1)
                        nc.sync.dma_start(wk_t, wk[m])
                        for n in range(4):
                            sl_n = slice(512 * n, 512 * (n + 1))
                            ps_k = ps2.tile([P, 512], F32, tag="ps_k")
                            for kc in range(KD):
                                nc.tensor.matmul(ps_k, wk_t[:, kc], h[:, kc, sl_n],
                                                 start=(kc == 0), stop=(kc == KD - 1))
                            kf = p2.tile([P, 512], F32, tag="kf", bufs=2)
                            nc.vector.tensor_copy(kf, ps_k)
                            ps_rot = ps2.tile([P, 512], F32, tag="ps_rot")
                            nc.tensor.matmul(ps_rot, perm_t, kf, start=True, stop=True)
                            tc_c = p2.tile([P, 512], F32, tag="tc_c", bufs=2)
                            nc.vector.tensor_tensor(tc_c, kf, cosf_t[:, sl_n], MUL)
                            ts_s = p2.tile([P, 512], F32, tag="ts_s", bufs=2)
                            nc.vector.tensor_tensor(ts_s, ps_rot, sinf_t[:, sl_n], MUL)
                            kr = p2.tile([P, 512], MM_DT, tag="kr", bufs=2)
                            nc.vector.tensor_tensor(kr, tc_c, ts_s, ADD)
                            nc.sync.dma_start(k_in[P * m:P * (m + 1), sl_n], kr)

                # ================= phase 3: V projection -> v_in =================
                with (
                    tc.tile_pool(name="p3", bufs=1) as p3,
                    tc.tile_pool(name="ps3", bufs=2, space="PSUM") as ps3,
                ):
                    wv_t = p3.tile([P, KD, DL], MM_DT)
                    nc.sync.dma_start(wv_t, wv[:, :])
                    for m in range(KD):
                        ps_v = ps3.tile([P, DL], F32, tag="ps_v")
                        for kc in range(KD):
                            nc.tensor.matmul(ps_v, h[:, kc, P * m:P * (m + 1)], wv_t[:, kc],
                                             start=(kc == 0), stop=(kc == KD - 1))
                        v_sb = p3.tile([P, DL], MM_DT, tag="v_sb", bufs=3)
                        nc.vector.tensor_copy(v_sb, ps_v)
                        nc.sync.dma_start(v_in[P * m:P * (m + 1), :], v_sb)

            # h / xh freed here. Launch the all-gathers.
            if single_core:
                # offline timeline-sim variant: fake the gather by replication
                for rr in range(NCORES):
                    nc.sync.dma_start(k_out[DL * rr:DL * (rr + 1), :], k_in[:, :])
                    nc.sync.dma_start(v_out[S * rr:S * (rr + 1), :], v_in[:, :])
            else:
                nc.gpsimd.collective_compute(
                    "AllGather", mybir.AluOpType.bypass,
                    replica_groups=[list(range(NCORES))],
                    ins=[k_in.opt()], outs=[k_out.opt()],
                )
                nc.gpsimd.collective_compute(
                    "AllGather", mybir.AluOpType.bypass,
                    replica_groups=[list(range(NCORES))],
                    ins=[v_in.opt()], outs=[v_out.opt()],
                )

            with tc.tile_pool(name="mid", bufs=1) as mid:
                xl_t = mid.tile([P, KD, SL], F32)
                nc.sync.dma_start(xl_t, xl[:, :])
                at = mid.tile([P, KD, SL], MM_DT)

                # ============ phase 4: local norm, Q projection + rope ============
                with (
                    tc.tile_pool(name="p4", bufs=1) as p4,
                    tc.tile_pool(name="ps4", bufs=2, space="PSUM") as ps4,
                ):
                    cosl_t = p4.tile([P, SL], F32)
                    sinl_t = p4.tile([P, SL], F32)
                    nc.sync.dma_start(cosl_t, cosl[:, :])
                    nc.sync.dma_start(sinl_t, sinl[:, :])
                    ps_msl = ps4.tile([1, SL], F32, bufs=1)
                    for kc in range(KD):
                        sql = p4.tile([P, SL], BF16, tag="sql", bufs=2)
                        nc.scalar.square(sql, xl_t[:, kc])
                        nc.tensor.matmul(ps_msl, ones_bf, sql,
                                         start=(kc == 0), stop=(kc == KD - 1))
                    rstdl = p4.tile([1, SL], F32)
                    nc.scalar.activation(rstdl, ps_msl, AF.Identity, bias=eps_t[0:1], scale=1.0 / D)
                    nc.scalar.sqrt(rstdl, rstdl)
                    nc.vector.reciprocal(rstdl, rstdl)
                    rbcl = p4.tile([P, SL], F32)
                    nc.gpsimd.partition_broadcast(rbcl, rstdl)
                    hl = p4.tile([P, KD, SL], MM_DT)
                    for kc in range(KD):
                        nc.vector.tensor_tensor(hl[:, kc], xl_t[:, kc], rbcl, MUL)

                    for hh in range(H):
                        wq_t = p4.tile([P, KD, HD], MM_DT, tag="wq", bufs=2)
                        nc.sync.dma_start(wq_t, wq[hh])
                        ps_q = ps4.tile([HD, SL], F32, tag="ps_q")
                        for kc in range(KD):
                            nc.tensor.matmul(ps_q, wq_t[:, kc], hl[:, kc],
                                             start=(kc == 0), stop=(kc == KD - 1))
                        qf = p4.tile([HD, SL], F32, tag="qf", bufs=2)
                        nc.vector.tensor_copy(qf, ps_q)
                        ps_rot = ps4.tile([HD, SL], F32, tag="ps_rotq")
                        nc.tensor.matmul(ps_rot, perm_t[0:HD, 0:HD], qf, start=True, stop=True)
                        tc_q = p4.tile([HD, SL], F32, tag="tc_q", bufs=2)
                        nc.vector.tensor_tensor(tc_q, qf, cosl_t[0:HD], MUL)
                        ts_q = p4.tile([HD, SL], F32, tag="ts_q", bufs=2)
                        nc.vector.tensor_tensor(ts_q, ps_rot, sinl_t[0:HD], MUL)
                        nc.vector.tensor_tensor(qTr[:, hh], tc_q, ts_q, ADD)

                # ============ phase 5: attention over all head pairs ============
                with (
                    tc.tile_pool(name="p5", bufs=1) as p5,
                    tc.tile_pool(name="ps5", bufs=1, space="PSUM") as ps5,
                ):
                    # zero-padded V tiles (double-buffered): head A lives in free
                    # cols 0:64, head B in 64:128 so its PV matmul writes psum
                    # partitions 64:127 with dst base 0 (nonzero dst base is
                    # rejected by walrus).
                    vpads = []
                    for i in range(4):
                        vpad_i = p5.tile([P, KD, P], MM_DT, name=f"vpad{i}")
                        nc.vector.tensor_scalar_mul(vpad_i, xl_t[:, :, 0:P], 0.0)
                        vpads.append(vpad_i)
                    for pp in range(NPAIR):
                        kpA = p5.tile([HD, S], MM_DT, tag="kpA", bufs=1)
                        nc.sync.dma_start(kpA, k_out[P * pp:P * pp + HD, :])
                        kpB = p5.tile([HD, S], MM_DT, tag="kpB", bufs=1)
                        nc.sync.dma_start(kpB, k_out[P * pp + HD:P * (pp + 1), :])
                        r, qq = pp // 2, pp % 2
                        vpA = vpads[2 * (pp % 2)]
                        vpB = vpads[2 * (pp % 2) + 1]
                        nc.sync.dma_start(
                            vpA[:, :, 0:64],
                            v_out[S * r:S * (r + 1), P * qq:P * qq + 64]
                            .rearrange("(kc p) f -> p kc f", p=P),
                        )
                        nc.sync.dma_start(
                            vpB[:, :, 64:128],
                            v_out[S * r:S * (r + 1), P * qq + 64:P * (qq + 1)]
                            .rearrange("(kc p) f -> p kc f", p=P),
                        )
                        E = p5.tile([P, KD, 2 * SL], MM_DT, tag="E", bufs=2)
                        for kc in range(KD):
                            ps_s = ps5.tile([P, 2 * SL], F32, tag="ps_s", bufs=3)
                            nc.tensor.matmul(ps_s[:, 0:SL], kpA[:, P * kc:P * (kc + 1)],
                                             qTr[:, 2 * pp], start=True, stop=True)
                            nc.tensor.matmul(ps_s[:, SL:2 * SL], kpB[:, P * kc:P * (kc + 1)],
                                             qTr[:, 2 * pp + 1], start=True, stop=True)
                            nc.scalar.activation(E[:, kc], ps_s, AF.Exp)
                        ps_d = ps5.tile([1, 2 * SL], F32, tag="ps_d", bufs=2)
                        for kc in range(KD):
                            nc.tensor.matmul(ps_d, ones_r, E[:, kc],
                                             start=(kc == 0), stop=(kc == KD - 1))
                        ps_av = ps5.tile([P, SL], F32, tag="ps_av", bufs=2)
                        for kc in range(KD):
                            nc.tensor.matmul(ps_av, vpA[:, kc], E[:, kc, 0:SL],
                                             start=(kc == 0), stop=False)
                            nc.tensor.matmul(ps_av, vpB[:, kc], E[:, kc, SL:2 * SL],
                                             start=False, stop=(kc == KD - 1))
                        recip = p5.tile([1, 2 * SL], F32, tag="recip", bufs=2)
                        nc.vector.reciprocal(recip, ps_d)
                        rbca = p5.tile([P, 2 * SL], F32, tag="rbca", bufs=2)
                        nc.gpsimd.partition_broadcast(rbca, recip)
                        nc.vector.tensor_tensor(at[0:64, pp], ps_av[0:64], rbca[0:64, 0:SL], MUL)
                        nc.vector.tensor_tensor(at[64:128, pp], ps_av[64:128],
                                                rbca[64:128, SL:2 * SL], MUL)

                with tc.tile_pool(name="late", bufs=1) as late:
                    x2 = late.tile([P, KD, SL], F32)
                    h2 = late.tile([P, KD, SL], MLP_DT)

                    # ============ phase 6: out-proj + residual, post-norm ============
                    with (
                        tc.tile_pool(name="p6", bufs=1) as p6,
                        tc.tile_pool(name="ps6", bufs=2, space="PSUM") as ps6,
                    ):
                        for m in range(KD):
                            wo_t = p6.tile([P, KD, P], MM_DT, tag="wo", bufs=2)
                            nc.sync.dma_start(wo_t, wo[m])
                            ps_o = ps6.tile([P, SL], F32, tag="ps_o")
                            for kc in range(KD):
                                nc.tensor.matmul(ps_o, wo_t[:, kc], at[:, kc],
                                                 start=(kc == 0), stop=(kc == KD - 1))
                            nc.vector.tensor_tensor(x2[:, m], ps_o, xl_t[:, m], ADD)

                        ps_ms2 = ps6.tile([1, SL], F32, bufs=1)
                        for m in range(KD):
                            sq2 = p6.tile([P, SL], BF16, tag="sq2", bufs=2)
                            nc.scalar.square(sq2, x2[:, m])
                            nc.tensor.matmul(ps_ms2, ones_bf, sq2,
                                             start=(m == 0), stop=(m == KD - 1))
                        rstd2 = p6.tile([1, SL], F32)
                        nc.scalar.activation(rstd2, ps_ms2, AF.Identity, bias=eps_t[0:1], scale=1.0 / D)
                        nc.scalar.sqrt(rstd2, rstd2)
                        nc.vector.reciprocal(rstd2, rstd2)
                        rbc2 = p6.tile([P, SL], F32)
                        nc.gpsimd.partition_broadcast(rbc2, rstd2)
                        for m in range(KD):
                            nc.vector.tensor_tensor(h2[:, m], x2[:, m], rbc2, MUL)

                    # ============ phase 7: MLP ============
                    with (
                        tc.tile_pool(name="p7", bufs=1) as p7,
                        tc.tile_pool(name="ps7", bufs=2, space="PSUM") as ps7,
                    ):
                        act = p7.tile([P, KF, SL], MLP_DT)
                        for mf in range(KF):
                            wg_t = p7.tile([P, KD, P], MLP_DT, tag="wmlp", bufs=4)
                            nc.sync.dma_start(wg_t, wg[mf])
                            wu_t = p7.tile([P, KD, P], MLP_DT, tag="wmlp", bufs=4)
                            nc.sync.dma_start(wu_t, wu[mf])
                            ps_g = ps7.tile([P, SL], F32, tag="ps_g")
                            ps_u = ps7.tile([P, SL], F32, tag="ps_u")
                            for kc in range(KD):
                                nc.tensor.matmul(ps_g, wg_t[:, kc], h2[:, kc],
                                                 start=(kc == 0), stop=(kc == KD - 1))
                            for kc in range(KD):
                                nc.tensor.matmul(ps_u, wu_t[:, kc], h2[:, kc],
                                                 start=(kc == 0), stop=(kc == KD - 1))
                            stmp = p7.tile([P, SL], F32, tag="stmp", bufs=2)
                            nc.scalar.activation(stmp, ps_u, AF.Silu)
                            nc.vector.tensor_tensor(act[:, mf], ps_g, stmp, MUL)

                        for md in range(KD):
                            ps_y = ps7.tile([P, SL], F32, tag="ps_y")
                            for qtr in range(4):
                                wd_t = p7.tile([P, KD, P], MLP_DT, tag="wmlp", bufs=4)
                                nc.sync.dma_start(wd_t, wd[md][:, KD * qtr:KD * (qtr + 1), :])
                                for kk in range(KD):
                                    nc.tensor.matmul(ps_y, wd_t[:, kk], act[:, KD * qtr + kk],
                                                     start=(qtr == 0 and kk == 0),
                                                     stop=(qtr == 3 and kk == KD - 1))
                            o_sb = p7.tile([P, SL], F32, tag="o_sb", bufs=3)
                            nc.vector.tensor_tensor(o_sb, ps_y, x2[:, md], ADD)
                            nc.sync.dma_start(out[P * md:P * (md + 1), :], o_sb)
    nc.compile()
    return nc


_NC_CACHE = {}


def _get_nc():
    if "nc" not in _NC_CACHE:
        _NC_CACHE["nc"] = _build()
    return _NC_CACHE["nc"]


def _rope_tables():
    inv_freq = (1.0 / (THETA ** (np.arange(0, HD, 2, dtype=np.float32) / HD))).astype(np.float32)
    pos = np.arange(S, dtype=np.float32)
    freqs = pos[:, None] * inv_freq[None, :]                  # [S, HD/2]
    emb = np.concatenate([freqs, freqs], axis=-1)             # [S, HD]
    cos = np.cos(emb).astype(np.float32)
    sin = np.sin(emb).astype(np.float32)
    cosT = cos.T                                              # [HD, S]
    sinT = sin.T.copy()
    sinT[0:HD // 2] *= -1.0                                   # sign folded for rotate_half
    return np.tile(cosT, (2, 1)), np.tile(sinT, (2, 1))       # [128, S] each


def _perm_matrix():
    # lhsT for rotate-half shift: out[m] = q[(m+32) % 64 within each 64 block]
    pm = np.zeros((P, P), np.float32)
    for m in range(P):
        blk = (m // HD) * HD
        src = blk + (m - blk + HD // 2) % HD
        pm[src, m] = 1.0
    return pm


def _tile_lhsT(wT, n_m):
    # [Kdim, Mdim] -> [m, p, kc, f] blocks for SBUF lhsT tiles
    Kdim, Mdim = wT.shape
    kc = Kdim // P
    return np.ascontiguousarray(
        wT.reshape(kc, P, n_m, Mdim // n_m).transpose(2, 1, 0, 3))


def _prep_in_maps(inputs):
    x = np.asarray(inputs["x"], np.float32)
    w_in = np.asarray(inputs["w_in_norm"], np.float32)
    wq = np.asarray(inputs["wq"], np.float32)
    wk = np.asarray(inputs["wk"], np.float32)
    wv = np.asarray(inputs["wv"], np.float32)
    wo = np.asarray(inputs["wo"], np.float32)
    w_post = np.asarray(inputs["w_post_norm"], np.float32)
    wg = np.asarray(inputs["wg"], np.float32)
    wu = np.asarray(inputs["wu"], np.float32)
    wd = np.asarray(inputs["wd"], np.float32)

    xT = np.ascontiguousarray(x[0].T)                         # [D, S]
    xt_sb = np.ascontiguousarray(xT.reshape(KD, P, S).transpose(1, 0, 2))  # [p, kc, s]

    scale = 1.0 / np.sqrt(HD)
    wq_eff = (wq * w_in[None, :]) * scale
    wq_sb = _tile_lhsT(np.ascontiguousarray(wq_eff.T), H)     # [32, 128, 16, 64]
    wk_eff = wk * w_in[None, :]
    wv_eff = wv * w_in[None, :]
    wo_sb = _tile_lhsT(np.ascontiguousarray(wo.T), KD)
    import ml_dtypes
    bf16 = ml_dtypes.bfloat16
    wg_sb = _tile_lhsT(np.ascontiguousarray((wg * w_post[None, :]).T), KF).astype(bf16)
    wu_sb = _tile_lhsT(np.ascontiguousarray((wu * w_post[None, :]).T), KF).astype(bf16)
    wd_sb = _tile_lhsT(np.ascontiguousarray(wd.T), KD).astype(bf16)       # [16, 128, 64, 128]

    cosT, sinT = _rope_tables()
    pm = _perm_matrix()

    nc = _get_nc()
    in_maps = []
    for c in range(NCORES):
        wkT_c = np.ascontiguousarray(wk_eff[c * DL:(c + 1) * DL, :].T)   # [D, 256]
        wk_sb = _tile_lhsT(wkT_c, 2)                                     # [2, 128, 16, 128]
        wvT_c = np.ascontiguousarray(wv_eff[c * DL:(c + 1) * DL, :].T)   # [D, 256]
        wv_sb = np.ascontiguousarray(wvT_c.reshape(KD, P, DL).transpose(1, 0, 2))  # [128, 16, 256]
        sl = slice(c * SL, (c + 1) * SL)
        in_maps.append({
            "xt": xt_sb,
            "xl": np.ascontiguousarray(xt_sb[:, :, sl]),
            "wq": wq_sb, "wk": wk_sb, "wv": wv_sb, "wo": wo_sb,
            "wg": wg_sb, "wu": wu_sb, "wd": wd_sb,
            "cosf": cosT, "sinf": sinT,
            "cosl": np.ascontiguousarray(cosT[:, sl]),
            "sinl": np.ascontiguousarray(sinT[:, sl]),
            "perm": pm,
        })
    return in_maps


def kernel(**inputs):
    nc = _get_nc()
    in_maps = _prep_in_maps(inputs)
    res = run_bass_kernel_spmd(
        nc, in_maps, core_ids=list(range(NCORES)),
        trace=bool(os.environ.get("KERNEL_TRACE")),
        tmpdir=os.environ.get("KERNEL_TRACE_DIR") or None,
    )
    _NC_CACHE["last_result"] = res

    full = np.empty((B, S, D), np.float32)
    for c in range(NCORES):
        full[0, c * SL:(c + 1) * SL, :] = res.results[c]["out"].T
    return full



# revision 8
# speedup vs baseline: 1.6460x; 1.6460x over previous
"""Llama decoder layer on 8 Trainium2 NeuronCores.

Strategy (sequence-parallel, transposed activation space, bf16 compute):
  - Activations live transposed on-chip: [feature, token]; contraction dims land
    on SBUF partitions so every matmul is a natural PE op. All matmul operands
    are bf16 (f32 PSUM accumulate); norms/residual combine in fp32.
  - Each core owns S/8 = 256 token columns end-to-end (attention rows + full-FF
    MLP). K/V projections are head-sharded (4 heads per core over all 2048
    tokens), RoPE'd locally, then AllGathered (bf16) so each core attends over
    all 32 heads for its 256 query columns.
  - rmsnorm rstd is folded into the rope tables (K/Q) and applied as a
    per-partition scale on V outputs, so the full x^T is never rescaled.
  - Scores for a head pair run as ONE matmul per k-chunk against a zero-padded
    paired Q layout; V carries a 65th ones-column so the PV matmul emits the
    softmax denominator for free.
"""
import os
import sys

sys.path.insert(0, "/opt/trn_rl_repo")

import numpy as np

import concourse.bacc as bacc
import concourse.mybir as mybir
import concourse.tile as tile
from concourse.bass_utils import run_bass_kernel_spmd

F32 = mybir.dt.float32
BF16 = mybir.dt.bfloat16
AF = mybir.ActivationFunctionType
MUL = mybir.AluOpType.mult
ADD = mybir.AluOpType.add

B, S, D, H = 1, 2048, 2048, 32
HD = D // H            # 64
FF = 8192
EPS = 1e-5
THETA = 10000.0
P = 128
NCORES = 8
SL = S // NCORES       # 256 local tokens
DL = D // NCORES       # 256 local K/V dims (4 heads)
KD = D // P            # 16 chunks of D
KF = FF // P           # 64 chunks of FF
NPAIR = H // 2         # 16 head pairs


def _build(single_core=False):
    nc = bacc.Bacc(None, target_bir_lowering=False,
                   num_devices=1 if single_core else NCORES)

    # ---- I/O ----
    xt = nc.dram_tensor("xt", [P, KD, S], BF16, kind="ExternalInput")       # x^T tiled
    xl = nc.dram_tensor("xl", [P, KD, SL], F32, kind="ExternalInput")       # local x^T columns
    xlb = nc.dram_tensor("xlb", [P, KD, SL], BF16, kind="ExternalInput")    # local x^T (bf16)
    cosl = nc.dram_tensor("cosl", [P, SL], F32, kind="ExternalInput")       # raw local rope
    sinl = nc.dram_tensor("sinl", [P, SL], F32, kind="ExternalInput")
    wq = nc.dram_tensor("wq", [NPAIR, P, KD, P], BF16, kind="ExternalInput")
    wk = nc.dram_tensor("wk", [2, P, KD, P], BF16, kind="ExternalInput")
    wv = nc.dram_tensor("wv", [P, KD, DL], BF16, kind="ExternalInput")      # rhs layout
    wo = nc.dram_tensor("wo", [KD, P, KD, P], BF16, kind="ExternalInput")
    wg = nc.dram_tensor("wg", [KF, P, KD, P], BF16, kind="ExternalInput")
    wu = nc.dram_tensor("wu", [KF, P, KD, P], BF16, kind="ExternalInput")
    wd = nc.dram_tensor("wd", [KD, P, KF, P], BF16, kind="ExternalInput")
    cosf = nc.dram_tensor("cosf", [P, S], F32, kind="ExternalInput")        # raw rope tables
    sinf = nc.dram_tensor("sinf", [P, S], F32, kind="ExternalInput")
    perm = nc.dram_tensor("perm", [P, P], BF16, kind="ExternalInput")       # rotate_half permutation
    out = nc.dram_tensor("out", [D, SL], F32, kind="ExternalOutput")        # out^T local columns

    with tile.TileContext(nc) as tc:
        with (
            tc.tile_pool(name="persist", bufs=1) as persist,
            tc.tile_pool(name="dram", bufs=1, space="DRAM") as dram,
        ):
            ones_bf = persist.tile([P, 1], BF16)
            nc.vector.memset(ones_bf, 1.0)
            perm_t = persist.tile([P, P], BF16)
            nc.sync.dma_start(perm_t, perm[:, :])
            eps_t = persist.tile([P, 1], F32)
            nc.vector.memset(eps_t, EPS)
            # rstd-scaled rope tables (bf16), built in phase 1
            cosS = persist.tile([P, S], BF16)
            sinS = persist.tile([P, S], BF16)
            # rstd in token-partition layout [p, m] = rstd[m*128+p] (for V scale)
            rstdT = persist.tile([P, KD], F32)
            # zero-padded paired-rope'd Q: pair pp has head 2pp at partitions
            # 0:64 x cols 0:SL, head 2pp+1 at partitions 64:128 x cols SL:2SL
            qpair = persist.tile([P, NPAIR, 2 * SL], BF16)
            nc.vector.memset(qpair, 0.0)
            xl_t = persist.tile([P, KD, SL], F32)
            nc.sync.dma_start(xl_t, xl[:, :])

            k_in = dram.tile([DL, S], BF16)
            v_in = dram.tile([S, DL], BF16)
            rstd_d = dram.tile([1, S], F32)
            kv_addr = "Local" if single_core else "Shared"
            k_out = dram.tile([NCORES * DL, S], BF16, addr_space=kv_addr)
            v_out = dram.tile([NCORES * S, DL], BF16, addr_space=kv_addr)

            with tc.tile_pool(name="big", bufs=1) as big:
                xh = big.tile([P, KD, S], BF16)   # x^T (bf16), unnormalized
                for i in range(8):
                    nc.sync.dma_start(xh[:, 2 * i:2 * (i + 1)], xt[:, 2 * i:2 * (i + 1)])

                # ============ phase 1: mean-square over features, rope tables ============
                with (
                    tc.tile_pool(name="p1", bufs=1) as p1,
                    tc.tile_pool(name="ps1", bufs=1, space="PSUM") as ps1,
                ):
                    cos_f = p1.tile([P, S], F32)
                    sin_f = p1.tile([P, S], F32)
                    nc.sync.dma_start(cos_f, cosf[:, :])
                    nc.sync.dma_start(sin_f, sinf[:, :])
                    ps_ms = [ps1.tile([1, 512], F32, name=f"ps_ms{n}") for n in range(4)]
                    for kc in range(KD):
                        sq = p1.tile([P, S], BF16, tag="sq", bufs=2)
                        nc.vector.tensor_tensor(sq, xh[:, kc], xh[:, kc], MUL)
                        for n in range(4):
                            nc.tensor.matmul(ps_ms[n], ones_bf, sq[:, 512 * n:512 * (n + 1)],
                                             start=(kc == 0), stop=(kc == KD - 1))
                    srstd = p1.tile([1, S], F32)
                    for n in range(4):
                        nc.scalar.activation(srstd[:, 512 * n:512 * (n + 1)], ps_ms[n],
                                             AF.Identity, bias=eps_t[0:1], scale=1.0 / D)
                    nc.scalar.sqrt(srstd, srstd)            # sqrt(ms+eps), [1, S]
                    sbc = p1.tile([P, S], F32)
                    nc.gpsimd.partition_broadcast(sbc, srstd)
                    rbc = p1.tile([P, S], F32)              # rstd broadcast to all partitions
                    nc.vector.reciprocal_approx_fast(rbc, sbc)
                    nc.sync.dma_start(rstd_d, rbc[0:1, :])
                    nc.sync.dma_start(rstdT, rstd_d.rearrange("o (m p) -> p (o m)", p=P))
                    nc.vector.tensor_tensor(cosS, cos_f, rbc, MUL)
                    nc.vector.tensor_tensor(sinS, sin_f, rbc, MUL)

                # ============ phase 2: K projection + rope -> k_in, gather ============
                with (
                    tc.tile_pool(name="p2", bufs=1) as p2,
                    tc.tile_pool(name="ps2", bufs=2, space="PSUM") as ps2,
                ):
                    for m in range(2):
                        wk_t = p2.tile([P, KD, P], BF16, tag="wk", bufs=2)
                        nc.sync.dma_start(wk_t, wk[m])
                        for n in range(4):
                            sl_n = slice(512 * n, 512 * (n + 1))
                            ps_k = ps2.tile([P, 512], F32, tag="ps_k")
                            for kc in range(KD):
                                nc.tensor.matmul(ps_k, wk_t[:, kc], xh[:, kc, sl_n],
                                                 start=(kc == 0), stop=(kc == KD - 1))
                            kf = p2.tile([P, 512], BF16, tag="kf", bufs=2)
                            nc.vector.tensor_copy(kf, ps_k)
                            ps_rot = ps2.tile([P, 512], F32, tag="ps_rot")
                            nc.tensor.matmul(ps_rot, perm_t, kf, start=True, stop=True)
                            tc_c = p2.tile([P, 512], F32, tag="tc_c", bufs=2)
                            nc.vector.tensor_tensor(tc_c, kf, cosS[:, sl_n], MUL)
                            ts_s = p2.tile([P, 512], F32, tag="ts_s", bufs=2)
                            nc.vector.tensor_tensor(ts_s, ps_rot, sinS[:, sl_n], MUL)
                            kr = p2.tile([P, 512], BF16, tag="kr", bufs=2)
                            nc.vector.tensor_tensor(kr, tc_c, ts_s, ADD)
                            nc.sync.dma_start(k_in[P * m:P * (m + 1), sl_n], kr)

                if single_core:
                    for rr in range(NCORES):
                        nc.sync.dma_start(k_out[DL * rr:DL * (rr + 1), :], k_in[:, :])
                else:
                    nc.gpsimd.collective_compute(
                        "AllGather", mybir.AluOpType.bypass,
                        replica_groups=[list(range(NCORES))],
                        ins=[k_in.opt()], outs=[k_out.opt()],
                    )

                # ============ phase 3: V projection (rstd-scaled) -> v_in, gather ============
                with (
                    tc.tile_pool(name="p3", bufs=1) as p3,
                    tc.tile_pool(name="ps3", bufs=3, space="PSUM") as ps3,
                ):
                    wv_t = p3.tile([P, KD, DL], BF16)
                    nc.sync.dma_start(wv_t, wv[:, :])
                    for m in range(KD):
                        ps_v = ps3.tile([P, DL], F32, tag="ps_v")
                        for kc in range(KD):
                            nc.tensor.matmul(ps_v, xh[:, kc, P * m:P * (m + 1)], wv_t[:, kc],
                                             start=(kc == 0), stop=(kc == KD - 1))
                        v_sb = p3.tile([P, DL], BF16, tag="v_sb", bufs=3)
                        nc.scalar.mul(v_sb, ps_v, rstdT[:, m:m + 1])
                        nc.sync.dma_start(v_in[P * m:P * (m + 1), :], v_sb)

                if single_core:
                    for rr in range(NCORES):
                        nc.sync.dma_start(v_out[S * rr:S * (rr + 1), :], v_in[:, :])
                else:
                    nc.gpsimd.collective_compute(
                        "AllGather", mybir.AluOpType.bypass,
                        replica_groups=[list(range(NCORES))],
                        ins=[v_in.opt()], outs=[v_out.opt()],
                    )

                # ============ phase 4: Q projection + rope into qpair ============
                with (
                    tc.tile_pool(name="p4", bufs=1) as p4,
                    tc.tile_pool(name="ps4", bufs=2, space="PSUM") as ps4,
                ):
                    xlb_t = p4.tile([P, KD, SL], BF16)
                    nc.sync.dma_start(xlb_t, xlb[:, :])
                    cosl_f = p4.tile([P, SL], F32)
                    sinl_f = p4.tile([P, SL], F32)
                    nc.sync.dma_start(cosl_f, cosl[:, :])
                    nc.sync.dma_start(sinl_f, sinl[:, :])
                    # local rstd (recomputed from the local slice; can't index the
                    # full-length rstd with a per-core offset in SPMD code)
                    ps_msl = ps4.tile([1, SL], F32, bufs=1)
                    for kc in range(KD):
                        sql = p4.tile([P, SL], BF16, tag="sql", bufs=2)
                        nc.vector.tensor_tensor(sql, xlb_t[:, kc], xlb_t[:, kc], MUL)
                        nc.tensor.matmul(ps_msl, ones_bf, sql,
                                         start=(kc == 0), stop=(kc == KD - 1))
                    srstdl = p4.tile([1, SL], F32)
                    nc.scalar.activation(srstdl, ps_msl, AF.Identity,
                                         bias=eps_t[0:1], scale=1.0 / D)
                    nc.scalar.sqrt(srstdl, srstdl)
                    sbcl = p4.tile([P, SL], F32)
                    nc.gpsimd.partition_broadcast(sbcl, srstdl)
                    rbcl = p4.tile([P, SL], F32)
                    nc.vector.reciprocal_approx_fast(rbcl, sbcl)
                    coslS = p4.tile([P, SL], BF16)
                    sinlS = p4.tile([P, SL], BF16)
                    nc.vector.tensor_tensor(coslS, cosl_f, rbcl, MUL)
                    nc.vector.tensor_tensor(sinlS, sinl_f, rbcl, MUL)

                    for pp in range(NPAIR):
                        wq_t = p4.tile([P, KD, P], BF16, tag="wq", bufs=2)
                        nc.sync.dma_start(wq_t, wq[pp])
                        ps_q = ps4.tile([P, SL], F32, tag="ps_q")
                        for kc in range(KD):
                            nc.tensor.matmul(ps_q, wq_t[:, kc], xlb_t[:, kc],
                                             start=(kc == 0), stop=(kc == KD - 1))
                        qf = p4.tile([P, SL], BF16, tag="qf", bufs=2)
                        nc.vector.tensor_copy(qf, ps_q)
                        ps_rotq = ps4.tile([P, SL], F32, tag="ps_rotq")
                        nc.tensor.matmul(ps_rotq, perm_t, qf, start=True, stop=True)
                        tc_q = p4.tile([P, SL], F32, tag="tc_q", bufs=2)
                        nc.vector.tensor_tensor(tc_q, qf, coslS, MUL)
                        ts_q = p4.tile([P, SL], F32, tag="ts_q", bufs=2)
                        nc.vector.tensor_tensor(ts_q, ps_rotq, sinlS, MUL)
                        nc.vector.tensor_tensor(qpair[0:HD, pp, 0:SL],
                                                tc_q[0:HD], ts_q[0:HD], ADD)
                        nc.vector.tensor_tensor(qpair[HD:P, pp, SL:2 * SL],
                                                tc_q[HD:P], ts_q[HD:P], ADD)

            # xh freed here. Attention + MLP below.
            with tc.tile_pool(name="mid", bufs=1) as mid:
                at = mid.tile([P, NPAIR, SL], BF16)      # attn out^T (pair-chunked)
                tmpB = mid.tile([HD, NPAIR, SL], BF16)   # head-B halves (partition base 0)

                # ============ phase 5: attention over head pairs ============
                with (
                    tc.tile_pool(name="p5", bufs=1) as p5,
                    tc.tile_pool(name="ps5", bufs=1, space="PSUM") as ps5,
                ):
                    # V tiles with a 65th ones-column (PV then emits the softmax
                    # denominator at psum partition 64); ping-pong pairs of tiles
                    vps = []
                    for i in range(4):
                        vp = p5.tile([P, KD, HD + 1], BF16, name=f"vp{i}")
                        nc.vector.memset(vp[:, :, HD:HD + 1], 1.0)
                        vps.append(vp)
                    for pp in range(NPAIR):
                        r, qq = pp // 2, pp % 2
                        kp = p5.tile([P, S], BF16, tag="kp", bufs=2)
                        nc.sync.dma_start(kp, k_out[P * pp:P * (pp + 1), :])
                        vpA = vps[2 * (pp % 2)]
                        vpB = vps[2 * (pp % 2) + 1]
                        nc.sync.dma_start(
                            vpA[:, :, 0:HD],
                            v_out[S * r:S * (r + 1), P * qq:P * qq + HD]
                            .rearrange("(kc p) f -> p kc f", p=P),
                        )
                        nc.sync.dma_start(
                            vpB[:, :, 0:HD],
                            v_out[S * r:S * (r + 1), P * qq + HD:P * (qq + 1)]
                            .rearrange("(kc p) f -> p kc f", p=P),
                        )
                        E = p5.tile([P, KD, 2 * SL], BF16, tag="E", bufs=2)
                        for g in range(KD // 2):   # score 2 k-chunks, then one big exp
                            ps_s = ps5.tile([P, 2, 2 * SL], F32, tag="ps_s", bufs=3)
                            for j in range(2):
                                kc = 2 * g + j
                                nc.tensor.matmul(ps_s[:, j], kp[:, P * kc:P * (kc + 1)],
                                                 qpair[:, pp], start=True, stop=True)
                            nc.scalar.activation(E[:, 2 * g:2 * (g + 1)], ps_s, AF.Exp)
                        ps_ab = ps5.tile([HD + 1, 2, SL], F32, tag="ps_ab", bufs=2)
                        for kc in range(KD):
                            nc.tensor.matmul(ps_ab[:, 0], vpA[:, kc], E[:, kc, 0:SL],
                                             start=(kc == 0), stop=(kc == KD - 1))
                        for kc in range(KD):
                            nc.tensor.matmul(ps_ab[:, 1], vpB[:, kc], E[:, kc, SL:2 * SL],
                                             start=(kc == 0), stop=(kc == KD - 1))
                        dn = p5.tile([1, 2 * SL], F32, tag="dn", bufs=2)
                        nc.vector.tensor_copy(dn[:, 0:SL], ps_ab[HD:HD + 1, 0])
                        nc.vector.tensor_copy(dn[:, SL:2 * SL], ps_ab[HD:HD + 1, 1])
                        rdn = p5.tile([1, 2 * SL], F32, tag="rdn", bufs=2)
                        nc.vector.reciprocal_approx_fast(rdn, dn)
                        rbca = p5.tile([P, 2 * SL], F32, tag="rbca", bufs=2)
                        nc.gpsimd.partition_broadcast(rbca, rdn)
                        nc.vector.tensor_tensor(at[0:HD, pp], ps_ab[0:HD, 0],
                                                rbca[0:HD, 0:SL], MUL)
                        nc.vector.tensor_tensor(tmpB[:, pp], ps_ab[0:HD, 1],
                                                rbca[0:HD, SL:2 * SL], MUL)
                    # shift all head-B halves to partitions 64:128 in one DMA
                    nc.sync.dma_start(at[HD:P, :, :], tmpB[:, :, :])

                with tc.tile_pool(name="late", bufs=1) as late:
                    x2 = late.tile([P, KD, SL], F32)
                    h2 = late.tile([P, KD, SL], BF16)

                    # ============ phase 6: out-proj + residual, post-norm ============
                    with (
                        tc.tile_pool(name="p6", bufs=1) as p6,
                        tc.tile_pool(name="ps6", bufs=2, space="PSUM") as ps6,
                    ):
                        for m in range(KD):
                            wo_t = p6.tile([P, KD, P], BF16, tag="wo", bufs=2)
                            nc.sync.dma_start(wo_t, wo[m])
                            ps_o = ps6.tile([P, SL], F32, tag="ps_o")
                            for kc in range(KD):
                                nc.tensor.matmul(ps_o, wo_t[:, kc], at[:, kc],
                                                 start=(kc == 0), stop=(kc == KD - 1))
                            nc.vector.tensor_tensor(x2[:, m], ps_o, xl_t[:, m], ADD)

                        ps_ms2 = ps6.tile([1, SL], F32, bufs=1)
                        for m in range(KD):
                            sq2 = p6.tile([P, SL], BF16, tag="sq2", bufs=2)
                            nc.vector.tensor_tensor(sq2, x2[:, m], x2[:, m], MUL)
                            nc.tensor.matmul(ps_ms2, ones_bf, sq2,
                                             start=(m == 0), stop=(m == KD - 1))
                        srstd2 = p6.tile([1, SL], F32)
                        nc.scalar.activation(srstd2, ps_ms2, AF.Identity,
                                             bias=eps_t[0:1], scale=1.0 / D)
                        nc.scalar.sqrt(srstd2, srstd2)
                        sbc2 = p6.tile([P, SL], F32)
                        nc.gpsimd.partition_broadcast(sbc2, srstd2)
                        rbc2 = p6.tile([P, SL], F32)
                        nc.vector.reciprocal_approx_fast(rbc2, sbc2)
                        for m in range(KD):
                            nc.vector.tensor_tensor(h2[:, m], x2[:, m], rbc2, MUL)

                    # ============ phase 7: MLP ============
                    with (
                        tc.tile_pool(name="p7", bufs=1) as p7,
                        tc.tile_pool(name="ps7", bufs=2, space="PSUM") as ps7,
                    ):
                        act = p7.tile([P, KF, SL], BF16)
                        for mf in range(KF):
                            wg_t = p7.tile([P, KD, P], BF16, tag="wmlp", bufs=6)
                            nc.sync.dma_start(wg_t, wg[mf])
                            wu_t = p7.tile([P, KD, P], BF16, tag="wmlp", bufs=6)
                            nc.sync.dma_start(wu_t, wu[mf])
                            ps_g = ps7.tile([P, SL], F32, tag="ps_g")
                            ps_u = ps7.tile([P, SL], F32, tag="ps_u")
                            for kc in range(KD):
                                nc.tensor.matmul(ps_g, wg_t[:, kc], h2[:, kc],
                                                 start=(kc == 0), stop=(kc == KD - 1))
                            for kc in range(KD):
                                nc.tensor.matmul(ps_u, wu_t[:, kc], h2[:, kc],
                                                 start=(kc == 0), stop=(kc == KD - 1))
                            stmp = p7.tile([P, SL], F32, tag="stmp", bufs=2)
                            nc.scalar.activation(stmp, ps_u, AF.Silu)
                            nc.vector.tensor_tensor(act[:, mf], ps_g, stmp, MUL)

                        for md in range(KD):
                            ps_y = ps7.tile([P, SL], F32, tag="ps_y")
                            for qtr in range(4):
                                wd_t = p7.tile([P, KD, P], BF16, tag="wmlp", bufs=6)
                                nc.sync.dma_start(wd_t, wd[md][:, KD * qtr:KD * (qtr + 1), :])
                                for kk in range(KD):
                                    nc.tensor.matmul(ps_y, wd_t[:, kk], act[:, KD * qtr + kk],
                                                     start=(qtr == 0 and kk == 0),
                                                     stop=(qtr == 3 and kk == KD - 1))
                            o_sb = p7.tile([P, SL], F32, tag="o_sb", bufs=3)
                            nc.vector.tensor_tensor(o_sb, ps_y, x2[:, md], ADD)
                            nc.sync.dma_start(out[P * md:P * (md + 1), :], o_sb)
    nc.compile()
    return nc


_NC_CACHE = {}


def _get_nc():
    if "nc" not in _NC_CACHE:
        _NC_CACHE["nc"] = _build()
    return _NC_CACHE["nc"]


def _rope_tables():
    inv_freq = (1.0 / (THETA ** (np.arange(0, HD, 2, dtype=np.float32) / HD))).astype(np.float32)
    pos = np.arange(S, dtype=np.float32)
    freqs = pos[:, None] * inv_freq[None, :]                  # [S, HD/2]
    emb = np.concatenate([freqs, freqs], axis=-1)             # [S, HD]
    cos = np.cos(emb).astype(np.float32)
    sin = np.sin(emb).astype(np.float32)
    cosT = cos.T                                              # [HD, S]
    sinT = sin.T.copy()
    sinT[0:HD // 2] *= -1.0                                   # sign folded for rotate_half
    return np.tile(cosT, (2, 1)), np.tile(sinT, (2, 1))       # [128, S] each


def _perm_matrix():
    # lhsT for rotate-half shift: out[m] = q[(m+32) % 64 within each 64 block]
    pm = np.zeros((P, P), np.float32)
    for m in range(P):
        blk = (m // HD) * HD
        src = blk + (m - blk + HD // 2) % HD
        pm[src, m] = 1.0
    return pm


def _tile_lhsT(wT, n_m):
    # [Kdim, Mdim] -> [m, p, kc, f] blocks for SBUF lhsT tiles
    Kdim, Mdim = wT.shape
    kc = Kdim // P
    return np.ascontiguousarray(
        wT.reshape(kc, P, n_m, Mdim // n_m).transpose(2, 1, 0, 3))


def _prep_in_maps(inputs):
    import ml_dtypes
    bf16 = ml_dtypes.bfloat16

    x = np.asarray(inputs["x"], np.float32)
    w_in = np.asarray(inputs["w_in_norm"], np.float32)
    wq = np.asarray(inputs["wq"], np.float32)
    wk = np.asarray(inputs["wk"], np.float32)
    wv = np.asarray(inputs["wv"], np.float32)
    wo = np.asarray(inputs["wo"], np.float32)
    w_post = np.asarray(inputs["w_post_norm"], np.float32)
    wg = np.asarray(inputs["wg"], np.float32)
    wu = np.asarray(inputs["wu"], np.float32)
    wd = np.asarray(inputs["wd"], np.float32)

    xT = np.ascontiguousarray(x[0].T)                         # [D, S]
    xt_f = np.ascontiguousarray(xT.reshape(KD, P, S).transpose(1, 0, 2))  # [p, kc, s]
    xt_sb = xt_f.astype(bf16)

    scale = 1.0 / np.sqrt(HD)
    wq_eff = (wq * w_in[None, :]) * scale
    wq_sb = _tile_lhsT(np.ascontiguousarray(wq_eff.T), NPAIR).astype(bf16)
    wk_eff = wk * w_in[None, :]
    wv_eff = wv * w_in[None, :]
    wo_sb = _tile_lhsT(np.ascontiguousarray(wo.T), KD).astype(bf16)
    wg_sb = _tile_lhsT(np.ascontiguousarray((wg * w_post[None, :]).T), KF).astype(bf16)
    wu_sb = _tile_lhsT(np.ascontiguousarray((wu * w_post[None, :]).T), KF).astype(bf16)
    wd_sb = _tile_lhsT(np.ascontiguousarray(wd.T), KD).astype(bf16)

    cosT, sinT = _rope_tables()
    pm = _perm_matrix().astype(bf16)

    nc = _get_nc()
    in_maps = []
    for c in range(NCORES):
        wkT_c = np.ascontiguousarray(wk_eff[c * DL:(c + 1) * DL, :].T)   # [D, 256]
        wk_sb = _tile_lhsT(wkT_c, 2).astype(bf16)                        # [2, 128, 16, 128]
        wvT_c = np.ascontiguousarray(wv_eff[c * DL:(c + 1) * DL, :].T)   # [D, 256]
        wv_sb = np.ascontiguousarray(
            wvT_c.reshape(KD, P, DL).transpose(1, 0, 2)).astype(bf16)    # [128, 16, 256]
        sl = slice(c * SL, (c + 1) * SL)
        xl_f = np.ascontiguousarray(xt_f[:, :, sl])
        in_maps.append({
            "xt": xt_sb,
            "xl": xl_f,
            "xlb": xl_f.astype(bf16),
            "wq": wq_sb, "wk": wk_sb, "wv": wv_sb, "wo": wo_sb,
            "wg": wg_sb, "wu": wu_sb, "wd": wd_sb,
            "cosf": cosT, "sinf": sinT,
            "cosl": np.ascontiguousarray(cosT[:, sl]),
            "sinl": np.ascontiguousarray(sinT[:, sl]),
            "perm": pm,
        })
    return in_maps


def kernel(**inputs):
    nc = _get_nc()
    in_maps = _prep_in_maps(inputs)
    res = run_bass_kernel_spmd(
        nc, in_maps, core_ids=list(range(NCORES)),
        trace=bool(os.environ.get("KERNEL_TRACE")),
        tmpdir=os.environ.get("KERNEL_TRACE_DIR") or None,
    )
    _NC_CACHE["last_result"] = res

    full = np.empty((B, S, D), np.float32)
    for c in range(NCORES):
        full[0, c * SL:(c + 1) * SL, :] = res.results[c]["out"].T
    return full


# revision 22
# speedup vs baseline: 1.7013x; 1.0336x over previous
"""Llama decoder layer on 8 Trainium2 NeuronCores.

Strategy (sequence-parallel, transposed activation space, bf16 compute):
  - Activations live transposed on-chip: [feature, token]; contraction dims land
    on SBUF partitions so every matmul is a natural PE op. All matmul operands
    are bf16 (f32 PSUM accumulate); norms/residual combine in fp32.
  - Each core owns S/8 = 256 token columns end-to-end (attention rows + full-FF
    MLP). K/V projections are head-sharded (4 heads per core over all 2048
    tokens), RoPE'd locally, then AllGathered (bf16) so each core attends over
    all 32 heads for its 256 query columns.
  - rmsnorm rstd is folded into the rope tables (K/Q) and applied as a
    per-partition scale on V outputs, so the full x^T is never rescaled.
  - Scores for a head pair run as ONE matmul per k-chunk against a zero-padded
    paired Q layout; V carries a 65th ones-column so the PV matmul emits the
    softmax denominator for free.
"""
import os
import sys

sys.path.insert(0, "/opt/trn_rl_repo")

import numpy as np

import concourse.bacc as bacc
import concourse.mybir as mybir
import concourse.tile as tile
from concourse.bass_utils import run_bass_kernel_spmd

F32 = mybir.dt.float32
BF16 = mybir.dt.bfloat16
AF = mybir.ActivationFunctionType
MUL = mybir.AluOpType.mult
ADD = mybir.AluOpType.add

B, S, D, H = 1, 2048, 2048, 32
HD = D // H            # 64
FF = 8192
EPS = 1e-5
THETA = 10000.0
P = 128
NCORES = 8
SL = S // NCORES       # 256 local tokens
DL = D // NCORES       # 256 local K/V dims (4 heads)
KD = D // P            # 16 chunks of D
KF = FF // P           # 64 chunks of FF
NPAIR = H // 2         # 16 head pairs


def _build(single_core=False):
    nc = bacc.Bacc(None, target_bir_lowering=False,
                   num_devices=1 if single_core else NCORES)

    # ---- I/O ----
    xt = nc.dram_tensor("xt", [P, KD, S], BF16, kind="ExternalInput")       # x^T tiled
    xl = nc.dram_tensor("xl", [P, KD, SL], F32, kind="ExternalInput")       # local x^T columns
    xlb = nc.dram_tensor("xlb", [P, KD, SL], BF16, kind="ExternalInput")    # local x^T (bf16)
    cosl = nc.dram_tensor("cosl", [P, SL], BF16, kind="ExternalInput")      # raw local rope
    sinl = nc.dram_tensor("sinl", [P, SL], BF16, kind="ExternalInput")
    wq = nc.dram_tensor("wq", [P, NPAIR, KD, P], BF16, kind="ExternalInput")
    wk = nc.dram_tensor("wk", [2, P, KD, P], BF16, kind="ExternalInput")
    wv = nc.dram_tensor("wv", [P, KD, DL], BF16, kind="ExternalInput")      # rhs layout
    wo = nc.dram_tensor("wo", [KD, P, KD, P], BF16, kind="ExternalInput")
    wg = nc.dram_tensor("wg", [KF, P, KD, P], BF16, kind="ExternalInput")
    wu = nc.dram_tensor("wu", [KF, P, KD, P], BF16, kind="ExternalInput")
    wd = nc.dram_tensor("wd", [KD, P, KF, P], BF16, kind="ExternalInput")
    cosf = nc.dram_tensor("cosf", [P, S], BF16, kind="ExternalInput")       # raw rope tables
    sinf = nc.dram_tensor("sinf", [P, S], BF16, kind="ExternalInput")
    perm = nc.dram_tensor("perm", [P, P], BF16, kind="ExternalInput")       # rotate_half permutation
    out = nc.dram_tensor("out", [D, SL], F32, kind="ExternalOutput")        # out^T local columns

    with tile.TileContext(nc) as tc:
        with (
            tc.tile_pool(name="persist", bufs=1) as persist,
            tc.tile_pool(name="dram", bufs=1, space="DRAM") as dram,
        ):
            ones_bf = persist.tile([P, 1], BF16)
            nc.vector.memset(ones_bf, 1.0)
            perm_t = persist.tile([P, P], BF16)
            nc.sync.dma_start(perm_t, perm[:, :])
            eps_t = persist.tile([P, 1], F32)
            nc.vector.memset(eps_t, EPS)
            # rstd-scaled rope tables (bf16), built in phase 1
            cosS = persist.tile([P, S], BF16)
            sinS = persist.tile([P, S], BF16)
            # rstd in token-partition layout [p, m] = rstd[m*128+p] (for V scale)
            rstdT = persist.tile([P, KD], F32)
            # zero-padded paired-rope'd Q: pair pp has head 2pp at partitions
            # 0:64 x cols 0:SL, head 2pp+1 at partitions 64:128 x cols SL:2SL
            qpair = persist.tile([P, NPAIR, 2 * SL], BF16)
            nc.vector.memset(qpair, 0.0)

            k_in = dram.tile([DL, S], BF16)
            v_in = dram.tile([S, DL], BF16)
            rstd_d = dram.tile([1, S], F32)
            kv_addr = "Local" if single_core else "Shared"
            k_out = dram.tile([NCORES * DL, S], BF16, addr_space=kv_addr)
            v_out = dram.tile([NCORES * S, DL], BF16, addr_space=kv_addr)

            with tc.tile_pool(name="big", bufs=1) as big:
                xh = big.tile([P, KD, S], BF16)   # x^T (bf16), unnormalized
                for i in range(8):
                    nc.sync.dma_start(xh[:, 2 * i:2 * (i + 1)], xt[:, 2 * i:2 * (i + 1)])
                # preload ALL Q weights during the early idle-DMA window so the
                # Q phase never competes with the K/V AllGather HBM traffic
                wq_all = big.tile([P, NPAIR, KD, P], BF16)
                for i in range(4):
                    nc.sync.dma_start(wq_all[:, 4 * i:4 * (i + 1)], wq[:, 4 * i:4 * (i + 1)])
                nc.sync.dma_start(cosS, cosf[:, :])
                nc.sync.dma_start(sinS, sinf[:, :])

                # ============ phase 1: mean-square over features, rope tables ============
                with (
                    tc.tile_pool(name="p1", bufs=1) as p1,
                    tc.tile_pool(name="ps1", bufs=1, space="PSUM") as ps1,
                ):
                    ps_ms = [ps1.tile([1, 512], F32, name=f"ps_ms{n}") for n in range(4)]
                    for kc in range(KD):
                        sq = p1.tile([P, S], BF16, tag="sq", bufs=2)
                        nc.vector.tensor_tensor(sq, xh[:, kc], xh[:, kc], MUL)
                        for n in range(4):
                            nc.tensor.matmul(ps_ms[n], ones_bf, sq[:, 512 * n:512 * (n + 1)],
                                             start=(kc == 0), stop=(kc == KD - 1))
                    srstd = p1.tile([1, S], F32)
                    for n in range(4):
                        nc.scalar.activation(srstd[:, 512 * n:512 * (n + 1)], ps_ms[n],
                                             AF.Identity, bias=eps_t[0:1], scale=1.0 / D)
                    nc.scalar.sqrt(srstd, srstd)            # sqrt(ms+eps), [1, S]
                    rrstd = p1.tile([1, S], F32)
                    nc.vector.reciprocal_approx_fast(rrstd, srstd)
                    nc.sync.dma_start(rstd_d, rrstd)
                    nc.sync.dma_start(rstdT, rstd_d.rearrange("o (m p) -> p (o m)", p=P))
                    rbc = p1.tile([P, S], F32)              # rstd broadcast to all partitions
                    nc.gpsimd.partition_broadcast(rbc, rrstd)
                    nc.vector.tensor_tensor(cosS, cosS, rbc, MUL)
                    nc.vector.tensor_tensor(sinS, sinS, rbc, MUL)

                # ============ phase 2: K projection + rope -> k_in, gather ============
                with (
                    tc.tile_pool(name="p2", bufs=1) as p2,
                    tc.tile_pool(name="ps2", bufs=2, space="PSUM") as ps2,
                ):
                    for m in range(2):
                        wk_t = p2.tile([P, KD, P], BF16, tag="wk", bufs=2)
                        nc.sync.dma_start(wk_t, wk[m])
                        for n in range(4):
                            sl_n = slice(512 * n, 512 * (n + 1))
                            ps_k = ps2.tile([P, 512], F32, tag="ps_k")
                            for kc in range(KD):
                                nc.tensor.matmul(ps_k, wk_t[:, kc], xh[:, kc, sl_n],
                                                 start=(kc == 0), stop=(kc == KD - 1))
                            kf = p2.tile([P, 512], BF16, tag="kf", bufs=2)
                            nc.vector.tensor_copy(kf, ps_k)
                            ps_rot = ps2.tile([P, 512], F32, tag="ps_rot")
                            nc.tensor.matmul(ps_rot, perm_t, kf, start=True, stop=True)
                            tc_c = p2.tile([P, 512], F32, tag="tc_c", bufs=2)
                            nc.vector.tensor_tensor(tc_c, kf, cosS[:, sl_n], MUL)
                            ts_s = p2.tile([P, 512], F32, tag="ts_s", bufs=2)
                            nc.vector.tensor_tensor(ts_s, ps_rot, sinS[:, sl_n], MUL)
                            kr = p2.tile([P, 512], BF16, tag="kr", bufs=2)
                            nc.vector.tensor_tensor(kr, tc_c, ts_s, ADD)
                            nc.sync.dma_start(k_in[P * m:P * (m + 1), sl_n], kr)

                if single_core:
                    for rr in range(NCORES):
                        nc.sync.dma_start(k_out[DL * rr:DL * (rr + 1), :], k_in[:, :])
                else:
                    nc.gpsimd.collective_compute(
                        "AllGather", mybir.AluOpType.bypass,
                        replica_groups=[list(range(NCORES))],
                        ins=[k_in.opt()], outs=[k_out.opt()],
                    )

                # ============ phase 3: V projection (rstd-scaled) -> v_in, gather ============
                with (
                    tc.tile_pool(name="p3", bufs=1) as p3,
                    tc.tile_pool(name="ps3", bufs=3, space="PSUM") as ps3,
                ):
                    wv_t = p3.tile([P, KD, DL], BF16)
                    nc.sync.dma_start(wv_t, wv[:, :])
                    for m in range(KD):
                        ps_v = ps3.tile([P, DL], F32, tag="ps_v")
                        for kc in range(KD):
                            nc.tensor.matmul(ps_v, xh[:, kc, P * m:P * (m + 1)], wv_t[:, kc],
                                             start=(kc == 0), stop=(kc == KD - 1))
                        v_sb = p3.tile([P, DL], BF16, tag="v_sb", bufs=3)
                        nc.scalar.mul(v_sb, ps_v, rstdT[:, m:m + 1])
                        nc.sync.dma_start(v_in[P * m:P * (m + 1), :], v_sb)

                if single_core:
                    for rr in range(NCORES):
                        nc.sync.dma_start(v_out[S * rr:S * (rr + 1), :], v_in[:, :])
                else:
                    nc.gpsimd.collective_compute(
                        "AllGather", mybir.AluOpType.bypass,
                        replica_groups=[list(range(NCORES))],
                        ins=[v_in.opt()], outs=[v_out.opt()],
                    )

                # ============ phase 4: Q projection + rope into qpair ============
                with (
                    tc.tile_pool(name="p4", bufs=1) as p4,
                    tc.tile_pool(name="ps4", bufs=2, space="PSUM") as ps4,
                ):
                    xlb_t = p4.tile([P, KD, SL], BF16)
                    nc.sync.dma_start(xlb_t, xlb[:, :])
                    coslS = p4.tile([P, SL], BF16)
                    sinlS = p4.tile([P, SL], BF16)
                    nc.sync.dma_start(coslS, cosl[:, :])
                    nc.sync.dma_start(sinlS, sinl[:, :])
                    # local rstd (recomputed from the local slice; can't index the
                    # full-length rstd with a per-core offset in SPMD code)
                    ps_msl = ps4.tile([1, SL], F32, bufs=1)
                    for kc in range(KD):
                        sql = p4.tile([P, SL], BF16, tag="sql", bufs=2)
                        nc.vector.tensor_tensor(sql, xlb_t[:, kc], xlb_t[:, kc], MUL)
                        nc.tensor.matmul(ps_msl, ones_bf, sql,
                                         start=(kc == 0), stop=(kc == KD - 1))
                    srstdl = p4.tile([1, SL], F32)
                    nc.scalar.activation(srstdl, ps_msl, AF.Identity,
                                         bias=eps_t[0:1], scale=1.0 / D)
                    nc.scalar.sqrt(srstdl, srstdl)
                    rrstdl = p4.tile([1, SL], F32)
                    nc.vector.reciprocal_approx_fast(rrstdl, srstdl)
                    rbcl = p4.tile([P, SL], F32)
                    nc.gpsimd.partition_broadcast(rbcl, rrstdl)
                    nc.vector.tensor_tensor(coslS, coslS, rbcl, MUL)
                    nc.vector.tensor_tensor(sinlS, sinlS, rbcl, MUL)

                    for pp in range(NPAIR):
                        ps_q = ps4.tile([P, SL], F32, tag="ps_q")
                        for kc in range(KD):
                            nc.tensor.matmul(ps_q, wq_all[:, pp, kc], xlb_t[:, kc],
                                             start=(kc == 0), stop=(kc == KD - 1))
                        qf = p4.tile([P, SL], BF16, tag="qf", bufs=2)
                        nc.vector.tensor_copy(qf, ps_q)
                        ps_rotq = ps4.tile([P, SL], F32, tag="ps_rotq")
                        nc.tensor.matmul(ps_rotq, perm_t, qf, start=True, stop=True)
                        tc_q = p4.tile([P, SL], F32, tag="tc_q", bufs=2)
                        nc.vector.tensor_tensor(tc_q, qf, coslS, MUL)
                        ts_q = p4.tile([P, SL], F32, tag="ts_q", bufs=2)
                        nc.vector.tensor_tensor(ts_q, ps_rotq, sinlS, MUL)
                        nc.vector.tensor_tensor(qpair[0:HD, pp, 0:SL],
                                                tc_q[0:HD], ts_q[0:HD], ADD)
                        nc.vector.tensor_tensor(qpair[HD:P, pp, SL:2 * SL],
                                                tc_q[HD:P], ts_q[HD:P], ADD)

            # xh freed here. Attention + MLP below.
            with tc.tile_pool(name="mid", bufs=1) as mid:
                at = mid.tile([P, NPAIR, SL], BF16)      # attn out^T (pair-chunked)
                tmpB = mid.tile([HD, NPAIR, SL], BF16)   # head-B halves (partition base 0)
                xl_t = mid.tile([P, KD, SL], F32)        # residual (f32)
                nc.sync.dma_start(xl_t, xl[:, :])

                # ============ phase 5: attention over head pairs ============
                with (
                    tc.tile_pool(name="p5", bufs=1) as p5,
                    tc.tile_pool(name="ps5", bufs=1, space="PSUM") as ps5,
                ):
                    # V tiles with a 65th ones-column (PV then emits the softmax
                    # denominator at psum partition 64); ping-pong pairs of tiles
                    vps = []
                    for i in range(4):
                        vp = p5.tile([P, KD, HD + 1], BF16, name=f"vp{i}")
                        nc.vector.memset(vp[:, :, HD:HD + 1], 1.0)
                        vps.append(vp)
                    for pp in range(NPAIR):
                        r, qq = pp // 2, pp % 2
                        kp = p5.tile([P, S], BF16, tag="kp", bufs=2)
                        nc.sync.dma_start(kp, k_out[P * pp:P * (pp + 1), :])
                        vpA = vps[2 * (pp % 2)]
                        vpB = vps[2 * (pp % 2) + 1]
                        nc.sync.dma_start(
                            vpA[:, :, 0:HD],
                            v_out[S * r:S * (r + 1), P * qq:P * qq + HD]
                            .rearrange("(kc p) f -> p kc f", p=P),
                        )
                        nc.sync.dma_start(
                            vpB[:, :, 0:HD],
                            v_out[S * r:S * (r + 1), P * qq + HD:P * (qq + 1)]
                            .rearrange("(kc p) f -> p kc f", p=P),
                        )
                        E = p5.tile([P, KD, 2 * SL], BF16, tag="E", bufs=3)
                        for g in range(KD // 2):   # score 2 k-chunks, then one big exp
                            ps_s = ps5.tile([P, 2, 2 * SL], F32, tag="ps_s", bufs=3)
                            for j in range(2):
                                kc = 2 * g + j
                                nc.tensor.matmul(ps_s[:, j], kp[:, P * kc:P * (kc + 1)],
                                                 qpair[:, pp], start=True, stop=True)
                            nc.scalar.activation(E[:, 2 * g:2 * (g + 1)], ps_s, AF.Exp)
                        ps_ab = ps5.tile([HD + 1, 2, SL], F32, tag="ps_ab", bufs=2)
                        for kc in range(KD):
                            nc.tensor.matmul(ps_ab[:, 0], vpA[:, kc], E[:, kc, 0:SL],
                                             start=(kc == 0), stop=(kc == KD - 1))
                        for kc in range(KD):
                            nc.tensor.matmul(ps_ab[:, 1], vpB[:, kc], E[:, kc, SL:2 * SL],
                                             start=(kc == 0), stop=(kc == KD - 1))
                        dn = p5.tile([1, 2 * SL], F32, tag="dn", bufs=2)
                        nc.vector.tensor_copy(dn[:, 0:SL], ps_ab[HD:HD + 1, 0])
                        nc.vector.tensor_copy(dn[:, SL:2 * SL], ps_ab[HD:HD + 1, 1])
                        rdn = p5.tile([1, 2 * SL], F32, tag="rdn", bufs=2)
                        nc.vector.reciprocal_approx_fast(rdn, dn)
                        rbca = p5.tile([P, 2 * SL], F32, tag="rbca", bufs=2)
                        nc.gpsimd.partition_broadcast(rbca, rdn)
                        nc.vector.tensor_tensor(at[0:HD, pp], ps_ab[0:HD, 0],
                                                rbca[0:HD, 0:SL], MUL)
                        nc.vector.tensor_tensor(tmpB[:, pp], ps_ab[0:HD, 1],
                                                rbca[0:HD, SL:2 * SL], MUL)
                    # shift all head-B halves to partitions 64:128 in one DMA
                    nc.sync.dma_start(at[HD:P, :, :], tmpB[:, :, :])

                with tc.tile_pool(name="late", bufs=1) as late:
                    x2 = late.tile([P, KD, SL], F32)
                    h2 = late.tile([P, KD, SL], BF16)

                    # ============ phase 6: out-proj + residual, post-norm ============
                    with (
                        tc.tile_pool(name="p6", bufs=1) as p6,
                        tc.tile_pool(name="ps6", bufs=2, space="PSUM") as ps6,
                    ):
                        for m in range(KD):
                            wo_t = p6.tile([P, KD, P], BF16, tag="wo", bufs=4)
                            nc.sync.dma_start(wo_t, wo[m])
                            ps_o = ps6.tile([P, SL], F32, tag="ps_o")
                            for kc in range(KD):
                                nc.tensor.matmul(ps_o, wo_t[:, kc], at[:, kc],
                                                 start=(kc == 0), stop=(kc == KD - 1))
                            nc.vector.tensor_tensor(x2[:, m], ps_o, xl_t[:, m], ADD)

                        ps_ms2 = ps6.tile([1, SL], F32, bufs=1)
                        for m in range(KD):
                            sq2 = p6.tile([P, SL], BF16, tag="sq2", bufs=2)
                            nc.vector.tensor_tensor(sq2, x2[:, m], x2[:, m], MUL)
                            nc.tensor.matmul(ps_ms2, ones_bf, sq2,
                                             start=(m == 0), stop=(m == KD - 1))
                        srstd2 = p6.tile([1, SL], F32)
                        nc.scalar.activation(srstd2, ps_ms2, AF.Identity,
                                             bias=eps_t[0:1], scale=1.0 / D)
                        nc.scalar.sqrt(srstd2, srstd2)
                        sbc2 = p6.tile([P, SL], F32)
                        nc.gpsimd.partition_broadcast(sbc2, srstd2)
                        rbc2 = p6.tile([P, SL], F32)
                        nc.vector.reciprocal_approx_fast(rbc2, sbc2)
                        for m in range(KD):
                            nc.vector.tensor_tensor(h2[:, m], x2[:, m], rbc2, MUL)

                    # ============ phase 7: MLP ============
                    with (
                        tc.tile_pool(name="p7", bufs=1) as p7,
                        tc.tile_pool(name="ps7", bufs=2, space="PSUM") as ps7,
                    ):
                        act = p7.tile([P, KF, SL], BF16)
                        for mf in range(KF):
                            wg_t = p7.tile([P, KD, P], BF16, tag="wmlp", bufs=6)
                            nc.sync.dma_start(wg_t, wg[mf])
                            wu_t = p7.tile([P, KD, P], BF16, tag="wmlp", bufs=6)
                            nc.sync.dma_start(wu_t, wu[mf])
                            ps_g = ps7.tile([P, SL], F32, tag="ps_g")
                            ps_u = ps7.tile([P, SL], F32, tag="ps_u")
                            for kc in range(KD):
                                nc.tensor.matmul(ps_g, wg_t[:, kc], h2[:, kc],
                                                 start=(kc == 0), stop=(kc == KD - 1))
                            for kc in range(KD):
                                nc.tensor.matmul(ps_u, wu_t[:, kc], h2[:, kc],
                                                 start=(kc == 0), stop=(kc == KD - 1))
                            stmp = p7.tile([P, SL], F32, tag="stmp", bufs=2)
                            nc.scalar.activation(stmp, ps_u, AF.Silu)
                            nc.vector.tensor_tensor(act[:, mf], ps_g, stmp, MUL)

                        for md in range(KD):
                            ps_y = ps7.tile([P, SL], F32, tag="ps_y")
                            for qtr in range(4):
                                wd_t = p7.tile([P, KD, P], BF16, tag="wmlp", bufs=6)
                                nc.sync.dma_start(wd_t, wd[md][:, KD * qtr:KD * (qtr + 1), :])
                                for kk in range(KD):
                                    nc.tensor.matmul(ps_y, wd_t[:, kk], act[:, KD * qtr + kk],
                                                     start=(qtr == 0 and kk == 0),
                                                     stop=(qtr == 3 and kk == KD - 1))
                            o_sb = p7.tile([P, SL], F32, tag="o_sb", bufs=3)
                            nc.vector.tensor_tensor(o_sb, ps_y, x2[:, md], ADD)
                            nc.sync.dma_start(out[P * md:P * (md + 1), :], o_sb)
    nc.compile()
    return nc


_NC_CACHE = {}


def _get_nc():
    if "nc" not in _NC_CACHE:
        _NC_CACHE["nc"] = _build()
    return _NC_CACHE["nc"]


def _rope_tables():
    inv_freq = (1.0 / (THETA ** (np.arange(0, HD, 2, dtype=np.float32) / HD))).astype(np.float32)
    pos = np.arange(S, dtype=np.float32)
    freqs = pos[:, None] * inv_freq[None, :]                  # [S, HD/2]
    emb = np.concatenate([freqs, freqs], axis=-1)             # [S, HD]
    cos = np.cos(emb).astype(np.float32)
    sin = np.sin(emb).astype(np.float32)
    cosT = cos.T                                              # [HD, S]
    sinT = sin.T.copy()
    sinT[0:HD // 2] *= -1.0                                   # sign folded for rotate_half
    return np.tile(cosT, (2, 1)), np.tile(sinT, (2, 1))       # [128, S] each


def _perm_matrix():
    # lhsT for rotate-half shift: out[m] = q[(m+32) % 64 within each 64 block]
    pm = np.zeros((P, P), np.float32)
    for m in range(P):
        blk = (m // HD) * HD
        src = blk + (m - blk + HD // 2) % HD
        pm[src, m] = 1.0
    return pm


def _tile_lhsT(wT, n_m):
    # [Kdim, Mdim] -> [m, p, kc, f] blocks for SBUF lhsT tiles
    Kdim, Mdim = wT.shape
    kc = Kdim // P
    return np.ascontiguousarray(
        wT.reshape(kc, P, n_m, Mdim // n_m).transpose(2, 1, 0, 3))


def _prep_in_maps(inputs):
    import ml_dtypes
    bf16 = ml_dtypes.bfloat16

    x = np.asarray(inputs["x"], np.float32)
    w_in = np.asarray(inputs["w_in_norm"], np.float32)
    wq = np.asarray(inputs["wq"], np.float32)
    wk = np.asarray(inputs["wk"], np.float32)
    wv = np.asarray(inputs["wv"], np.float32)
    wo = np.asarray(inputs["wo"], np.float32)
    w_post = np.asarray(inputs["w_post_norm"], np.float32)
    wg = np.asarray(inputs["wg"], np.float32)
    wu = np.asarray(inputs["wu"], np.float32)
    wd = np.asarray(inputs["wd"], np.float32)

    xT = np.ascontiguousarray(x[0].T)                         # [D, S]
    xt_f = np.ascontiguousarray(xT.reshape(KD, P, S).transpose(1, 0, 2))  # [p, kc, s]
    xt_sb = xt_f.astype(bf16)

    scale = 1.0 / np.sqrt(HD)
    wq_eff = (wq * w_in[None, :]) * scale
    wq_sb = np.ascontiguousarray(
        _tile_lhsT(np.ascontiguousarray(wq_eff.T), NPAIR)
        .transpose(1, 0, 2, 3)).astype(bf16)                 # [p, pair, kc, f]
    wk_eff = wk * w_in[None, :]
    wv_eff = wv * w_in[None, :]
    wo_sb = _tile_lhsT(np.ascontiguousarray(wo.T), KD).astype(bf16)
    wg_sb = _tile_lhsT(np.ascontiguousarray((wg * w_post[None, :]).T), KF).astype(bf16)
    wu_sb = _tile_lhsT(np.ascontiguousarray((wu * w_post[None, :]).T), KF).astype(bf16)
    wd_sb = _tile_lhsT(np.ascontiguousarray(wd.T), KD).astype(bf16)

    cosT, sinT = _rope_tables()
    cosT_b = cosT.astype(bf16)
    sinT_b = sinT.astype(bf16)
    pm = _perm_matrix().astype(bf16)

    nc = _get_nc()
    in_maps = []
    for c in range(NCORES):
        wkT_c = np.ascontiguousarray(wk_eff[c * DL:(c + 1) * DL, :].T)   # [D, 256]
        wk_sb = _tile_lhsT(wkT_c, 2).astype(bf16)                        # [2, 128, 16, 128]
        wvT_c = np.ascontiguousarray(wv_eff[c * DL:(c + 1) * DL, :].T)   # [D, 256]
        wv_sb = np.ascontiguousarray(
            wvT_c.reshape(KD, P, DL).transpose(1, 0, 2)).astype(bf16)    # [128, 16, 256]
        sl = slice(c * SL, (c + 1) * SL)
        xl_f = np.ascontiguousarray(xt_f[:, :, sl])
        in_maps.append({
            "xt": xt_sb,
            "xl": xl_f,
            "xlb": xl_f.astype(bf16),
            "wq": wq_sb, "wk": wk_sb, "wv": wv_sb, "wo": wo_sb,
            "wg": wg_sb, "wu": wu_sb, "wd": wd_sb,
            "cosf": cosT_b, "sinf": sinT_b,
            "cosl": np.ascontiguousarray(cosT_b[:, sl]),
            "sinl": np.ascontiguousarray(sinT_b[:, sl]),
            "perm": pm,
        })
    return in_maps


def kernel(**inputs):
    nc = _get_nc()
    in_maps = _prep_in_maps(inputs)
    res = run_bass_kernel_spmd(
        nc, in_maps, core_ids=list(range(NCORES)),
        trace=bool(os.environ.get("KERNEL_TRACE")),
        tmpdir=os.environ.get("KERNEL_TRACE_DIR") or None,
    )
    _NC_CACHE["last_result"] = res

    full = np.empty((B, S, D), np.float32)
    for c in range(NCORES):
        full[0, c * SL:(c + 1) * SL, :] = res.results[c]["out"].T
    return full


# revision 26
# speedup vs baseline: 1.7108x; 1.0055x over previous
"""Llama decoder layer on 8 Trainium2 NeuronCores.

Strategy (sequence-parallel, transposed activation space, bf16 compute):
  - Activations live transposed on-chip: [feature, token]; contraction dims land
    on SBUF partitions so every matmul is a natural PE op. All matmul operands
    are bf16 (f32 PSUM accumulate); norms/residual combine in fp32.
  - Each core owns S/8 = 256 token columns end-to-end (attention rows + full-FF
    MLP). K/V projections are head-sharded (4 heads per core over all 2048
    tokens), RoPE'd locally, then AllGathered (bf16) so each core attends over
    all 32 heads for its 256 query columns.
  - rmsnorm rstd is folded into the rope tables (K/Q) and applied as a
    per-partition scale on V outputs, so the full x^T is never rescaled.
  - Scores for a head pair run as ONE matmul per k-chunk against a zero-padded
    paired Q layout; V carries a 65th ones-column so the PV matmul emits the
    softmax denominator for free.
"""
import os
import sys

sys.path.insert(0, "/opt/trn_rl_repo")

import numpy as np

import concourse.bacc as bacc
import concourse.mybir as mybir
import concourse.tile as tile
from concourse.bass_utils import run_bass_kernel_spmd

F32 = mybir.dt.float32
BF16 = mybir.dt.bfloat16
AF = mybir.ActivationFunctionType
MUL = mybir.AluOpType.mult
ADD = mybir.AluOpType.add

B, S, D, H = 1, 2048, 2048, 32
HD = D // H            # 64
FF = 8192
EPS = 1e-5
THETA = 10000.0
P = 128
NCORES = 8
SL = S // NCORES       # 256 local tokens
DL = D // NCORES       # 256 local K/V dims (4 heads)
KD = D // P            # 16 chunks of D
KF = FF // P           # 64 chunks of FF
NPAIR = H // 2         # 16 head pairs


def _build(single_core=False):
    nc = bacc.Bacc(None, target_bir_lowering=False,
                   num_devices=1 if single_core else NCORES)

    # ---- I/O ----
    xt = nc.dram_tensor("xt", [P, KD, S], BF16, kind="ExternalInput")       # x^T tiled
    xl = nc.dram_tensor("xl", [P, KD, SL], F32, kind="ExternalInput")       # local x^T columns
    xlb = nc.dram_tensor("xlb", [P, KD, SL], BF16, kind="ExternalInput")    # local x^T (bf16)
    cosl = nc.dram_tensor("cosl", [P, SL], BF16, kind="ExternalInput")      # raw local rope
    sinl = nc.dram_tensor("sinl", [P, SL], BF16, kind="ExternalInput")
    wq = nc.dram_tensor("wq", [P, NPAIR, KD, P], BF16, kind="ExternalInput")
    wk = nc.dram_tensor("wk", [2, P, KD, P], BF16, kind="ExternalInput")
    wv = nc.dram_tensor("wv", [P, KD, DL], BF16, kind="ExternalInput")      # rhs layout
    wo = nc.dram_tensor("wo", [KD, P, KD, P], BF16, kind="ExternalInput")
    wg = nc.dram_tensor("wg", [KF, P, KD, P], BF16, kind="ExternalInput")
    wu = nc.dram_tensor("wu", [KF, P, KD, P], BF16, kind="ExternalInput")
    wd = nc.dram_tensor("wd", [KD, P, KF, P], BF16, kind="ExternalInput")
    cosf = nc.dram_tensor("cosf", [P, S], BF16, kind="ExternalInput")       # raw rope tables
    sinf = nc.dram_tensor("sinf", [P, S], BF16, kind="ExternalInput")
    perm = nc.dram_tensor("perm", [P, P], BF16, kind="ExternalInput")       # rotate_half permutation
    out = nc.dram_tensor("out", [D, SL], F32, kind="ExternalOutput")        # out^T local columns

    with tile.TileContext(nc) as tc:
        with (
            tc.tile_pool(name="persist", bufs=1) as persist,
            tc.tile_pool(name="dram", bufs=1, space="DRAM") as dram,
        ):
            ones_bf = persist.tile([P, 1], BF16)
            nc.vector.memset(ones_bf, 1.0)
            perm_t = persist.tile([P, P], BF16)
            nc.sync.dma_start(perm_t, perm[:, :])
            eps_t = persist.tile([P, 1], F32)
            nc.vector.memset(eps_t, EPS)
            # rstd-scaled rope tables (bf16), built in phase 1
            cosS = persist.tile([P, S], BF16)
            sinS = persist.tile([P, S], BF16)
            # rstd in token-partition layout [p, m] = rstd[m*128+p] (for V scale)
            rstdT = persist.tile([P, KD], F32)
            # zero-padded paired-rope'd Q: pair pp has head 2pp at partitions
            # 0:64 x cols 0:SL, head 2pp+1 at partitions 64:128 x cols SL:2SL
            qpair = persist.tile([P, NPAIR, 2 * SL], BF16)
            nc.vector.memset(qpair, 0.0)

            k_in = dram.tile([DL, S], BF16)
            v_in = dram.tile([S, DL], BF16)
            rstd_d = dram.tile([1, S], F32)
            kv_addr = "Local" if single_core else "Shared"
            k_out = dram.tile([NCORES * DL, S], BF16, addr_space=kv_addr)
            v_out = dram.tile([NCORES * S, DL], BF16, addr_space=kv_addr)

            with tc.tile_pool(name="big", bufs=1) as big:
                xh = big.tile([P, KD, S], BF16)   # x^T (bf16), unnormalized
                for i in range(8):
                    nc.sync.dma_start(xh[:, 2 * i:2 * (i + 1)], xt[:, 2 * i:2 * (i + 1)])
                # K/V weights right behind x so phase 2/3 start as soon as the
                # norm is done; Q weights preloaded before the AllGathers start
                # so the Q phase never competes with gather HBM traffic
                wk_all = big.tile([P, 2, KD, P], BF16)
                for m in range(2):
                    nc.sync.dma_start(wk_all[:, m], wk[m])
                wv_t = big.tile([P, KD, DL], BF16)
                nc.sync.dma_start(wv_t, wv[:, :])
                nc.sync.dma_start(cosS, cosf[:, :])
                nc.sync.dma_start(sinS, sinf[:, :])
                wq_all = big.tile([P, NPAIR, KD, P], BF16)
                for i in range(4):
                    nc.sync.dma_start(wq_all[:, 4 * i:4 * (i + 1)], wq[:, 4 * i:4 * (i + 1)])

                # ============ phase 1: mean-square over features, rope tables ============
                with (
                    tc.tile_pool(name="p1", bufs=1) as p1,
                    tc.tile_pool(name="ps1", bufs=1, space="PSUM") as ps1,
                ):
                    ps_ms = [ps1.tile([1, 512], F32, name=f"ps_ms{n}") for n in range(4)]
                    for kc in range(KD):
                        sq = p1.tile([P, S], BF16, tag="sq", bufs=2)
                        nc.vector.tensor_tensor(sq, xh[:, kc], xh[:, kc], MUL)
                        for n in range(4):
                            nc.tensor.matmul(ps_ms[n], ones_bf, sq[:, 512 * n:512 * (n + 1)],
                                             start=(kc == 0), stop=(kc == KD - 1))
                    srstd = p1.tile([1, S], F32)
                    for n in range(4):
                        nc.scalar.activation(srstd[:, 512 * n:512 * (n + 1)], ps_ms[n],
                                             AF.Identity, bias=eps_t[0:1], scale=1.0 / D)
                    nc.scalar.sqrt(srstd, srstd)            # sqrt(ms+eps), [1, S]
                    rrstd = p1.tile([1, S], F32)
                    nc.vector.reciprocal_approx_fast(rrstd, srstd)
                    nc.sync.dma_start(rstd_d, rrstd)
                    nc.sync.dma_start(rstdT, rstd_d.rearrange("o (m p) -> p (o m)", p=P))
                    rbc = p1.tile([P, S], F32)              # rstd broadcast to all partitions
                    nc.gpsimd.partition_broadcast(rbc, rrstd)
                    nc.vector.tensor_tensor(cosS, cosS, rbc, MUL)
                    nc.vector.tensor_tensor(sinS, sinS, rbc, MUL)

                # ============ phase 2: K projection + rope -> k_in, gather ============
                with (
                    tc.tile_pool(name="p2", bufs=1) as p2,
                    tc.tile_pool(name="ps2", bufs=2, space="PSUM") as ps2,
                ):
                    # software-pipelined: rot matmul of slice i-1 is emitted
                    # during the projection of slice i so the in-order PE queue
                    # never waits on the DVE psum->kf copy (keeps HAM warm)
                    kf_all = p2.tile([P, 2, 4, 512], BF16)

                    def k_rope(m, n):
                        sl_n = slice(512 * n, 512 * (n + 1))
                        ps_rot = ps2.tile([P, 512], F32, tag="ps_rot")
                        nc.tensor.matmul(ps_rot, perm_t, kf_all[:, m, n],
                                         start=True, stop=True)
                        tc_c = p2.tile([P, 512], F32, tag="tc_c", bufs=2)
                        nc.vector.tensor_tensor(tc_c, kf_all[:, m, n], cosS[:, sl_n], MUL)
                        ts_s = p2.tile([P, 512], F32, tag="ts_s", bufs=2)
                        nc.vector.tensor_tensor(ts_s, ps_rot, sinS[:, sl_n], MUL)
                        kr = p2.tile([P, 512], BF16, tag="kr", bufs=2)
                        nc.vector.tensor_tensor(kr, tc_c, ts_s, ADD)
                        nc.sync.dma_start(k_in[P * m:P * (m + 1), sl_n], kr)

                    slices = [(m, n) for m in range(2) for n in range(4)]
                    for i, (m, n) in enumerate(slices):
                        sl_n = slice(512 * n, 512 * (n + 1))
                        ps_k = ps2.tile([P, 512], F32, tag="ps_k")
                        for kc in range(KD):
                            nc.tensor.matmul(ps_k, wk_all[:, m, kc], xh[:, kc, sl_n],
                                             start=(kc == 0), stop=(kc == KD - 1))
                        nc.vector.tensor_copy(kf_all[:, m, n], ps_k)
                        if i > 0:
                            k_rope(*slices[i - 1])
                    k_rope(*slices[-1])

                if single_core:
                    for rr in range(NCORES):
                        nc.sync.dma_start(k_out[DL * rr:DL * (rr + 1), :], k_in[:, :])
                else:
                    nc.gpsimd.collective_compute(
                        "AllGather", mybir.AluOpType.bypass,
                        replica_groups=[list(range(NCORES))],
                        ins=[k_in.opt()], outs=[k_out.opt()],
                    )

                # ============ phase 3: V projection (rstd-scaled) -> v_in, gather ============
                with (
                    tc.tile_pool(name="p3", bufs=1) as p3,
                    tc.tile_pool(name="ps3", bufs=3, space="PSUM") as ps3,
                ):
                    for m in range(KD):
                        ps_v = ps3.tile([P, DL], F32, tag="ps_v")
                        for kc in range(KD):
                            nc.tensor.matmul(ps_v, xh[:, kc, P * m:P * (m + 1)], wv_t[:, kc],
                                             start=(kc == 0), stop=(kc == KD - 1))
                        v_sb = p3.tile([P, DL], BF16, tag="v_sb", bufs=3)
                        nc.scalar.mul(v_sb, ps_v, rstdT[:, m:m + 1])
                        nc.sync.dma_start(v_in[P * m:P * (m + 1), :], v_sb)

                if single_core:
                    for rr in range(NCORES):
                        nc.sync.dma_start(v_out[S * rr:S * (rr + 1), :], v_in[:, :])
                else:
                    nc.gpsimd.collective_compute(
                        "AllGather", mybir.AluOpType.bypass,
                        replica_groups=[list(range(NCORES))],
                        ins=[v_in.opt()], outs=[v_out.opt()],
                    )

                # ============ phase 4: Q projection + rope into qpair ============
                with (
                    tc.tile_pool(name="p4", bufs=1) as p4,
                    tc.tile_pool(name="ps4", bufs=2, space="PSUM") as ps4,
                ):
                    xlb_t = p4.tile([P, KD, SL], BF16)
                    nc.sync.dma_start(xlb_t, xlb[:, :])
                    coslS = p4.tile([P, SL], BF16)
                    sinlS = p4.tile([P, SL], BF16)
                    nc.sync.dma_start(coslS, cosl[:, :])
                    nc.sync.dma_start(sinlS, sinl[:, :])
                    # local rstd (recomputed from the local slice; can't index the
                    # full-length rstd with a per-core offset in SPMD code)
                    ps_msl = ps4.tile([1, SL], F32, bufs=1)
                    for kc in range(KD):
                        sql = p4.tile([P, SL], BF16, tag="sql", bufs=2)
                        nc.vector.tensor_tensor(sql, xlb_t[:, kc], xlb_t[:, kc], MUL)
                        nc.tensor.matmul(ps_msl, ones_bf, sql,
                                         start=(kc == 0), stop=(kc == KD - 1))
                    srstdl = p4.tile([1, SL], F32)
                    nc.scalar.activation(srstdl, ps_msl, AF.Identity,
                                         bias=eps_t[0:1], scale=1.0 / D)
                    nc.scalar.sqrt(srstdl, srstdl)
                    rrstdl = p4.tile([1, SL], F32)
                    nc.vector.reciprocal_approx_fast(rrstdl, srstdl)
                    rbcl = p4.tile([P, SL], F32)
                    nc.gpsimd.partition_broadcast(rbcl, rrstdl)
                    nc.vector.tensor_tensor(coslS, coslS, rbcl, MUL)
                    nc.vector.tensor_tensor(sinlS, sinlS, rbcl, MUL)

                    qf_all = p4.tile([P, NPAIR, SL], BF16)

                    def q_rope(pp):
                        ps_rotq = ps4.tile([P, SL], F32, tag="ps_rotq")
                        nc.tensor.matmul(ps_rotq, perm_t, qf_all[:, pp],
                                         start=True, stop=True)
                        tc_q = p4.tile([P, SL], F32, tag="tc_q", bufs=2)
                        nc.vector.tensor_tensor(tc_q, qf_all[:, pp], coslS, MUL)
                        ts_q = p4.tile([P, SL], F32, tag="ts_q", bufs=2)
                        nc.vector.tensor_tensor(ts_q, ps_rotq, sinlS, MUL)
                        nc.vector.tensor_tensor(qpair[0:HD, pp, 0:SL],
                                                tc_q[0:HD], ts_q[0:HD], ADD)
                        nc.vector.tensor_tensor(qpair[HD:P, pp, SL:2 * SL],
                                                tc_q[HD:P], ts_q[HD:P], ADD)

                    for pp in range(NPAIR):
                        ps_q = ps4.tile([P, SL], F32, tag="ps_q")
                        for kc in range(KD):
                            nc.tensor.matmul(ps_q, wq_all[:, pp, kc], xlb_t[:, kc],
                                             start=(kc == 0), stop=(kc == KD - 1))
                        nc.vector.tensor_copy(qf_all[:, pp], ps_q)
                        if pp > 0:
                            q_rope(pp - 1)
                    q_rope(NPAIR - 1)

            # xh freed here. Attention + MLP below.
            with tc.tile_pool(name="mid", bufs=1) as mid:
                at = mid.tile([P, NPAIR, SL], BF16)      # attn out^T (pair-chunked)
                tmpB = mid.tile([HD, NPAIR, SL], BF16)   # head-B halves (partition base 0)
                xl_t = mid.tile([P, KD, SL], F32)        # residual (f32)
                nc.sync.dma_start(xl_t, xl[:, :])

                # ============ phase 5: attention over head pairs ============
                with (
                    tc.tile_pool(name="p5", bufs=1) as p5,
                    tc.tile_pool(name="ps5", bufs=1, space="PSUM") as ps5,
                ):
                    # V tiles with a 65th ones-column (PV then emits the softmax
                    # denominator at psum partition 64); ping-pong pairs of tiles
                    vps = []
                    for i in range(4):
                        vp = p5.tile([P, KD, HD + 1], BF16, name=f"vp{i}")
                        nc.vector.memset(vp[:, :, HD:HD + 1], 1.0)
                        vps.append(vp)
                    for pp in range(NPAIR):
                        r, qq = pp // 2, pp % 2
                        kp = p5.tile([P, S], BF16, tag="kp", bufs=2)
                        nc.sync.dma_start(kp, k_out[P * pp:P * (pp + 1), :])
                        vpA = vps[2 * (pp % 2)]
                        vpB = vps[2 * (pp % 2) + 1]
                        nc.sync.dma_start(
                            vpA[:, :, 0:HD],
                            v_out[S * r:S * (r + 1), P * qq:P * qq + HD]
                            .rearrange("(kc p) f -> p kc f", p=P),
                        )
                        nc.sync.dma_start(
                            vpB[:, :, 0:HD],
                            v_out[S * r:S * (r + 1), P * qq + HD:P * (qq + 1)]
                            .rearrange("(kc p) f -> p kc f", p=P),
                        )
                        E = p5.tile([P, KD, 2 * SL], BF16, tag="E", bufs=3)
                        for g in range(KD // 2):   # score 2 k-chunks, then one big exp
                            ps_s = ps5.tile([P, 2, 2 * SL], F32, tag="ps_s", bufs=3)
                            for j in range(2):
                                kc = 2 * g + j
                                nc.tensor.matmul(ps_s[:, j], kp[:, P * kc:P * (kc + 1)],
                                                 qpair[:, pp], start=True, stop=True)
                            nc.scalar.activation(E[:, 2 * g:2 * (g + 1)], ps_s, AF.Exp)
                        ps_ab = ps5.tile([HD + 1, 2, SL], F32, tag="ps_ab", bufs=2)
                        for kc in range(KD):
                            nc.tensor.matmul(ps_ab[:, 0], vpA[:, kc], E[:, kc, 0:SL],
                                             start=(kc == 0), stop=(kc == KD - 1))
                        for kc in range(KD):
                            nc.tensor.matmul(ps_ab[:, 1], vpB[:, kc], E[:, kc, SL:2 * SL],
                                             start=(kc == 0), stop=(kc == KD - 1))
                        dn = p5.tile([1, 2 * SL], F32, tag="dn", bufs=2)
                        nc.vector.tensor_copy(dn[:, 0:SL], ps_ab[HD:HD + 1, 0])
                        nc.vector.tensor_copy(dn[:, SL:2 * SL], ps_ab[HD:HD + 1, 1])
                        rdn = p5.tile([1, 2 * SL], F32, tag="rdn", bufs=2)
                        nc.vector.reciprocal_approx_fast(rdn, dn)
                        rbca = p5.tile([P, 2 * SL], F32, tag="rbca", bufs=2)
                        nc.gpsimd.partition_broadcast(rbca, rdn)
                        nc.vector.tensor_tensor(at[0:HD, pp], ps_ab[0:HD, 0],
                                                rbca[0:HD, 0:SL], MUL)
                        nc.vector.tensor_tensor(tmpB[:, pp], ps_ab[0:HD, 1],
                                                rbca[0:HD, SL:2 * SL], MUL)
                    # shift all head-B halves to partitions 64:128 in one DMA
                    nc.sync.dma_start(at[HD:P, :, :], tmpB[:, :, :])

                with tc.tile_pool(name="late", bufs=1) as late:
                    x2 = late.tile([P, KD, SL], F32)
                    h2 = late.tile([P, KD, SL], BF16)

                    # ============ phase 6: out-proj + residual, post-norm ============
                    with (
                        tc.tile_pool(name="p6", bufs=1) as p6,
                        tc.tile_pool(name="ps6", bufs=2, space="PSUM") as ps6,
                    ):
                        for m in range(KD):
                            wo_t = p6.tile([P, KD, P], BF16, tag="wo", bufs=4)
                            nc.sync.dma_start(wo_t, wo[m])
                            ps_o = ps6.tile([P, SL], F32, tag="ps_o")
                            for kc in range(KD):
                                nc.tensor.matmul(ps_o, wo_t[:, kc], at[:, kc],
                                                 start=(kc == 0), stop=(kc == KD - 1))
                            nc.vector.tensor_tensor(x2[:, m], ps_o, xl_t[:, m], ADD)

                        ps_ms2 = ps6.tile([1, SL], F32, bufs=1)
                        for m in range(KD):
                            sq2 = p6.tile([P, SL], BF16, tag="sq2", bufs=2)
                            nc.vector.tensor_tensor(sq2, x2[:, m], x2[:, m], MUL)
                            nc.tensor.matmul(ps_ms2, ones_bf, sq2,
                                             start=(m == 0), stop=(m == KD - 1))
                        srstd2 = p6.tile([1, SL], F32)
                        nc.scalar.activation(srstd2, ps_ms2, AF.Identity,
                                             bias=eps_t[0:1], scale=1.0 / D)
                        nc.scalar.sqrt(srstd2, srstd2)
                        sbc2 = p6.tile([P, SL], F32)
                        nc.gpsimd.partition_broadcast(sbc2, srstd2)
                        rbc2 = p6.tile([P, SL], F32)
                        nc.vector.reciprocal_approx_fast(rbc2, sbc2)
                        for m in range(KD):
                            nc.vector.tensor_tensor(h2[:, m], x2[:, m], rbc2, MUL)

                    # ============ phase 7: MLP ============
                    with (
                        tc.tile_pool(name="p7", bufs=1) as p7,
                        tc.tile_pool(name="ps7", bufs=2, space="PSUM") as ps7,
                    ):
                        act = p7.tile([P, KF, SL], BF16)
                        for mf in range(KF):
                            wg_t = p7.tile([P, KD, P], BF16, tag="wmlp", bufs=6)
                            nc.sync.dma_start(wg_t, wg[mf])
                            wu_t = p7.tile([P, KD, P], BF16, tag="wmlp", bufs=6)
                            nc.sync.dma_start(wu_t, wu[mf])
                            ps_g = ps7.tile([P, SL], F32, tag="ps_g")
                            ps_u = ps7.tile([P, SL], F32, tag="ps_u")
                            for kc in range(KD):
                                nc.tensor.matmul(ps_g, wg_t[:, kc], h2[:, kc],
                                                 start=(kc == 0), stop=(kc == KD - 1))
                            for kc in range(KD):
                                nc.tensor.matmul(ps_u, wu_t[:, kc], h2[:, kc],
                                                 start=(kc == 0), stop=(kc == KD - 1))
                            stmp = p7.tile([P, SL], F32, tag="stmp", bufs=2)
                            nc.scalar.activation(stmp, ps_u, AF.Silu)
                            nc.vector.tensor_tensor(act[:, mf], ps_g, stmp, MUL)

                        for md in range(KD):
                            ps_y = ps7.tile([P, SL], F32, tag="ps_y")
                            for qtr in range(4):
                                wd_t = p7.tile([P, KD, P], BF16, tag="wmlp", bufs=6)
                                nc.sync.dma_start(wd_t, wd[md][:, KD * qtr:KD * (qtr + 1), :])
                                for kk in range(KD):
                                    nc.tensor.matmul(ps_y, wd_t[:, kk], act[:, KD * qtr + kk],
                                                     start=(qtr == 0 and kk == 0),
                                                     stop=(qtr == 3 and kk == KD - 1))
                            o_sb = p7.tile([P, SL], F32, tag="o_sb", bufs=3)
                            nc.vector.tensor_tensor(o_sb, ps_y, x2[:, md], ADD)
                            nc.sync.dma_start(out[P * md:P * (md + 1), :], o_sb)
    nc.compile()
    return nc


_NC_CACHE = {}


def _get_nc():
    if "nc" not in _NC_CACHE:
        _NC_CACHE["nc"] = _build()
    return _NC_CACHE["nc"]


def _rope_tables():
    inv_freq = (1.0 / (THETA ** (np.arange(0, HD, 2, dtype=np.float32) / HD))).astype(np.float32)
    pos = np.arange(S, dtype=np.float32)
    freqs = pos[:, None] * inv_freq[None, :]                  # [S, HD/2]
    emb = np.concatenate([freqs, freqs], axis=-1)             # [S, HD]
    cos = np.cos(emb).astype(np.float32)
    sin = np.sin(emb).astype(np.float32)
    cosT = cos.T                                              # [HD, S]
    sinT = sin.T.copy()
    sinT[0:HD // 2] *= -1.0                                   # sign folded for rotate_half
    return np.tile(cosT, (2, 1)), np.tile(sinT, (2, 1))       # [128, S] each


def _perm_matrix():
    # lhsT for rotate-half shift: out[m] = q[(m+32) % 64 within each 64 block]
    pm = np.zeros((P, P), np.float32)
    for m in range(P):
        blk = (m // HD) * HD
        src = blk + (m - blk + HD // 2) % HD
        pm[src, m] = 1.0
    return pm


def _tile_lhsT(wT, n_m):
    # [Kdim, Mdim] -> [m, p, kc, f] blocks for SBUF lhsT tiles
    Kdim, Mdim = wT.shape
    kc = Kdim // P
    return np.ascontiguousarray(
        wT.reshape(kc, P, n_m, Mdim // n_m).transpose(2, 1, 0, 3))


def _prep_in_maps(inputs):
    import ml_dtypes
    bf16 = ml_dtypes.bfloat16

    x = np.asarray(inputs["x"], np.float32)
    w_in = np.asarray(inputs["w_in_norm"], np.float32)
    wq = np.asarray(inputs["wq"], np.float32)
    wk = np.asarray(inputs["wk"], np.float32)
    wv = np.asarray(inputs["wv"], np.float32)
    wo = np.asarray(inputs["wo"], np.float32)
    w_post = np.asarray(inputs["w_post_norm"], np.float32)
    wg = np.asarray(inputs["wg"], np.float32)
    wu = np.asarray(inputs["wu"], np.float32)
    wd = np.asarray(inputs["wd"], np.float32)

    xT = np.ascontiguousarray(x[0].T)                         # [D, S]
    xt_f = np.ascontiguousarray(xT.reshape(KD, P, S).transpose(1, 0, 2))  # [p, kc, s]
    xt_sb = xt_f.astype(bf16)

    scale = 1.0 / np.sqrt(HD)
    wq_eff = (wq * w_in[None, :]) * scale
    wq_sb = np.ascontiguousarray(
        _tile_lhsT(np.ascontiguousarray(wq_eff.T), NPAIR)
        .transpose(1, 0, 2, 3)).astype(bf16)                 # [p, pair, kc, f]
    wk_eff = wk * w_in[None, :]
    wv_eff = wv * w_in[None, :]
    wo_sb = _tile_lhsT(np.ascontiguousarray(wo.T), KD).astype(bf16)
    wg_sb = _tile_lhsT(np.ascontiguousarray((wg * w_post[None, :]).T), KF).astype(bf16)
    wu_sb = _tile_lhsT(np.ascontiguousarray((wu * w_post[None, :]).T), KF).astype(bf16)
    wd_sb = _tile_lhsT(np.ascontiguousarray(wd.T), KD).astype(bf16)

    cosT, sinT = _rope_tables()
    cosT_b = cosT.astype(bf16)
    sinT_b = sinT.astype(bf16)
    pm = _perm_matrix().astype(bf16)

    nc = _get_nc()
    in_maps = []
    for c in range(NCORES):
        wkT_c = np.ascontiguousarray(wk_eff[c * DL:(c + 1) * DL, :].T)   # [D, 256]
        wk_sb = _tile_lhsT(wkT_c, 2).astype(bf16)                        # [2, 128, 16, 128]
        wvT_c = np.ascontiguousarray(wv_eff[c * DL:(c + 1) * DL, :].T)   # [D, 256]
        wv_sb = np.ascontiguousarray(
            wvT_c.reshape(KD, P, DL).transpose(1, 0, 2)).astype(bf16)    # [128, 16, 256]
        sl = slice(c * SL, (c + 1) * SL)
        xl_f = np.ascontiguousarray(xt_f[:, :, sl])
        in_maps.append({
            "xt": xt_sb,
            "xl": xl_f,
            "xlb": xl_f.astype(bf16),
            "wq": wq_sb, "wk": wk_sb, "wv": wv_sb, "wo": wo_sb,
            "wg": wg_sb, "wu": wu_sb, "wd": wd_sb,
            "cosf": cosT_b, "sinf": sinT_b,
            "cosl": np.ascontiguousarray(cosT_b[:, sl]),
            "sinl": np.ascontiguousarray(sinT_b[:, sl]),
            "perm": pm,
        })
    return in_maps


def kernel(**inputs):
    nc = _get_nc()
    in_maps = _prep_in_maps(inputs)
    res = run_bass_kernel_spmd(
        nc, in_maps, core_ids=list(range(NCORES)),
        trace=bool(os.environ.get("KERNEL_TRACE")),
        tmpdir=os.environ.get("KERNEL_TRACE_DIR") or None,
    )
    _NC_CACHE["last_result"] = res

    full = np.empty((B, S, D), np.float32)
    for c in range(NCORES):
        full[0, c * SL:(c + 1) * SL, :] = res.results[c]["out"].T
    return full
